# revision 4
# baseline (speedup 1.0000x reference)
"""ARMA-style GNN message passing on 8 TRN2 NeuronCores.

Reference computation (per layer, 7 layers):
    m   = h @ W                                  [N, CH]
    agg = segment_sum(w[:,None] * m[dst], src)   [N, CH]
    h'  = relu(agg + h @ V + b)
then logits = h @ Wd + bd.

Strategy (graph/data parallel over nodes, ReduceScatter formulation):
  - 8 cores own 1250 nodes each (padded to 1280 = 10 blocks of 128).
  - Edge (s, d) is processed by the core owning d (where the message row
    m[d] is LOCAL).  Each core computes, per layer, a partial aggregation
    table over the full padded-global src space [8*1280, CH]:
        partial_p[s] = sum_{e: dst_e in p, src_e = s} w_e * m_local[dst_e]
    then one ReduceScatter(add) sums the 8 partials and hands every core
    the [1280, CH] slice for its own nodes.  (A ReduceScatter's output is
    1/8 the size of the AllGather the src-side formulation needs, which is
    what the inter-chip collective cost scales with.)
  - Edges are bucketed by global src block (80 buckets of 128 src slots).
    Within a bucket the distinct local dst rows are deduplicated into
    gather tiles of 128 rows; one indirect DMA fetches a tile from the
    local m table in DRAM, and a [128 dst-rows x 128 src-slots] bf16
    "selection" matrix carrying the degree weights w_e scatter-adds it
    into the bucket's PSUM accumulator on PE.
  - Layer 1 scatters x itself (256 wide, half the traffic) and applies W1
    after the ReduceScatter: A@(x W1) == (A@x) W1.
  - h@V + b is precomputed into SBUF while the ReduceScatter is in
    flight; the post-collective epilogue is add + relu + PE-transpose.
  - Final dense layer and output assembly per core; host concatenates.

All matmuls run in bf16 with fp32 PSUM accumulation.
"""
import numpy as np
import ml_dtypes

import concourse.bass as bass
import concourse.tile as tile
import concourse.mybir as mybir
from concourse.vector_clock import ScopedClock
from concourse.bass_utils import run_bass_kernel_spmd
from concourse.masks import make_identity

# ---------------------------------------------------------------- constants
N_NODES = 10000
N_EDGES = 160000
IN_F = 256
CH = 512
N_LABELS = 1440
NCORES = 8
NPC = N_NODES // NCORES      # 1250 nodes per core
P = 128
NBL = 10                     # node blocks per core (10*128 = 1280)
NPAD = NBL * P               # padded nodes per core
NSB = NCORES * NBL           # global src blocks (80)
NGPAD = NCORES * NPAD        # padded global node space (10240)
NLAYERS = 7
KG1 = IN_F // P              # 2 contraction blocks in layer 1
KGC = CH // P                # 4 contraction blocks in layers 2..7
FIN_CHUNK = 480              # 1440 = 3 * 480, fits one PSUM bank in f32

BF = mybir.dt.bfloat16
F32 = mybir.dt.float32
BFNP = ml_dtypes.bfloat16


# ------------------------------------------------------- walrus workarounds
def _patched_drain_and_barrier(self, tick_clock, wait_clock):
    # This walrus build rejects >1-2 sync waits on one TPB_CTRL; put the
    # kernel-tail drain's waits on separate preceding SP nops instead.
    nc = self.nc
    probe = nc.sync.nop(nofuse=True, hint="drain_waits")
    wait_clock.add_sem_waits(probe.ins, ScopedClock({None: tick_clock.global_clock}))
    si = probe.ins.sync_info
    waits = list(si.on_wait) if si is not None else []
    if len(waits) > 1:
        si.on_wait = waits[:1]
        for i in range(1, len(waits)):
            n2 = nc.sync.nop(nofuse=True, hint=f"drain_waits_{i}")
            n2.ins.sync_info = mybir.SyncInfo(on_wait=[waits[i]], on_update=[])
    nc.sync.drain()
    nc.all_engine_barrier()
    assert self.sems is not None
    popped = nc._tile_sem_poison_stack.pop()
    assert popped is self._sem_poison
    nc.clear_and_free_semaphores(list(self.sems.allocated().values()))
    nc.all_engine_barrier()


tile.TileContext._drain_and_barrier = _patched_drain_and_barrier


def _split_excess_waits(nc, limit=1):
    # Same ISA restriction for ordinary instructions: hoist excess sync
    # waits onto injected same-engine nops placed just before.
    for func in nc.m.functions:
        for bb in func.blocks:
            out = []
            for ins in bb.instructions:
                si = ins.sync_info
                if si is not None and si.on_wait and len(si.on_wait) > limit:
                    waits = list(si.on_wait)
                    excess, keep = waits[:-limit], waits[-limit:]
                    for i in range(0, len(excess), limit):
                        out.append(mybir.InstNoOp(
                            name=f"{ins.name}_xw{i}",
                            engine=ins.engine,
                            ins=[], outs=[],
                            sync_info=mybir.SyncInfo(
                                on_wait=excess[i:i + limit], on_update=[]),
                        ))
                    si.on_wait = keep
                out.append(ins)
            bb.instructions[:] = out


# ------------------------------------------------------------- host prep
def _prep_edges(src, dst):
    """Route each edge to the core owning dst; bucket by global src block
    (80 buckets); dedupe distinct local dst rows into gather tiles of 128.
    Returns (ntiles, idx_tabs, sel_tabs): ntiles[b] = gather-tile count of
    bucket b (max over cores, same program on all cores); per core an
    idx table [128, NT] of local m-table rows and a sel table
    [128, NT*128] of bf16 degree weights (lane=dst row, col=src slot)."""
    src = np.asarray(src).astype(np.int64)
    dst = np.asarray(dst).astype(np.int64)
    deg_out = np.maximum(np.bincount(src, minlength=N_NODES), 1.0).astype(np.float32)
    deg_in = np.maximum(np.bincount(dst, minlength=N_NODES), 1.0).astype(np.float32)
    w = (1.0 / np.sqrt(deg_out[src] * deg_in[dst])).astype(np.float32)

    core = dst // NPC
    ldst = dst - core * NPC
    psrc = (src // NPC) * NPAD + (src % NPC)   # padded-global src row
    sb = psrc // P
    slot = psrc - sb * P

    order = np.lexsort((ldst, sb, core))
    c_s, sb_s = core[order], sb[order]
    ld_s, sl_s, w_s = ldst[order], slot[order], w[order]
    key = c_s * NSB + sb_s
    starts = np.searchsorted(key, np.arange(NCORES * NSB))
    ends = np.searchsorted(key, np.arange(NCORES * NSB), side="right")

    uniq_store = {}
    ntiles = np.ones(NSB, np.int64)
    for p in range(NCORES):
        for b in range(NSB):
            s0, s1 = starts[p * NSB + b], ends[p * NSB + b]
            if s1 > s0:
                uniq, inv = np.unique(ld_s[s0:s1], return_inverse=True)
            else:
                uniq, inv = np.zeros(1, np.int64), np.zeros(0, np.int64)
            uniq_store[p, b] = (uniq, inv, s0, s1)
            ntiles[b] = max(ntiles[b], -(-len(uniq) // P))
    colof = np.zeros(NSB, np.int64)
    colof[1:] = np.cumsum(ntiles)[:-1]
    nt = int(ntiles.sum())

    idx_tabs, sel_tabs = [], []
    for p in range(NCORES):
        idx_t = np.zeros((P, nt), np.int32)
        sel_t = np.zeros((P, nt * P), np.float32)
        for b in range(NSB):
            uniq, inv, s0, s1 = uniq_store[p, b]
            lanes_u = np.arange(len(uniq))
            idx_t[lanes_u % P, colof[b] + lanes_u // P] = uniq
            if s1 > s0:
                tile_e = inv // P
                lane_e = inv - tile_e * P
                np.add.at(sel_t, (lane_e, (colof[b] + tile_e) * P + sl_s[s0:s1]),
                          w_s[s0:s1])
        idx_tabs.append(idx_t)
        sel_tabs.append(sel_t.astype(BFNP))
    return list(ntiles), idx_tabs, sel_tabs


def _pack_lhsT(xT, kg):
    """[kg*128, NPAD] -> [128, kg*NPAD] (partition-major kg blocks)."""
    return np.ascontiguousarray(
        xT.reshape(kg, P, NPAD).transpose(1, 0, 2).reshape(P, kg * NPAD))


def _pack_rhs(Wm, kg, n):
    """[kg*128, n] -> [128, kg*n]."""
    return np.ascontiguousarray(
        Wm.reshape(kg, P, n).transpose(1, 0, 2).reshape(P, kg * n))


# ------------------------------------------------------------- device build
def _build(ntiles, repeat=1):
    nt = sum(ntiles)
    selsplit = sum(ntiles[:NSB // 2])   # cols in first sel half (load early)
    nc = bass.Bass("TRN2", target_bir_lowering=False, debug=False,
                   num_devices=NCORES)

    def din(name, shape, dt):
        return nc.dram_tensor(name, shape, dt, kind="ExternalInput").ap()

    xT = din("xT", [P, KG1 * NPAD], BF)
    xrows = din("xrows", [NPAD, IN_F], BF)
    idx = din("idx", [P, nt], mybir.dt.int32)
    sel = din("sel", [P, nt * P], BF)
    w1 = din("w1", [P, KG1 * CH], BF)
    v1 = din("v1", [P, KG1 * CH], BF)
    wk = din("wk", [P, 6 * KGC * CH], BF)
    vk = din("vk", [P, 6 * KGC * CH], BF)
    wd = din("wd", [P, KGC * N_LABELS], BF)
    ball = din("ball", [P, NLAYERS * CH], F32)
    bdr = din("bdr", [P, N_LABELS], F32)
    out = nc.dram_tensor("out", [NPAD, N_LABELS], F32, kind="ExternalOutput").ap()

    with tile.TileContext(nc) as tc:
        with (
            tc.tile_pool(name="const", bufs=1) as cp,
            tc.tile_pool(name="ht", bufs=2) as htp,
            tc.tile_pool(name="mout", bufs=3) as mp,
            tc.tile_pool(name="msg", bufs=8) as msgp,
            tc.tile_pool(name="pw", bufs=4) as pwp,
            tc.tile_pool(name="agg", bufs=4) as aggp,
            tc.tile_pool(name="hv", bufs=12) as hvp,
            tc.tile_pool(name="ep", bufs=3) as epp,
            tc.tile_pool(name="ut", bufs=2) as utp,
            tc.tile_pool(name="hact", bufs=2) as hp,
            tc.tile_pool(name="outs", bufs=2) as op,
            tc.tile_pool(name="psm", bufs=2, space="PSUM") as psm,
            tc.tile_pool(name="psagg", bufs=4, space="PSUM") as psagg,
            tc.tile_pool(name="pstr", bufs=2, space="PSUM") as pstr,
            tc.tile_pool(name="dram", bufs=1, space="DRAM") as dram,
        ):
            # ---- constants to SBUF, roughly in first-use order
            idx_t = cp.tile([P, nt], mybir.dt.int32)
            nc.sync.dma_start(idx_t[:], idx[:])
            sel_t = cp.tile([P, nt * P], BF)
            nc.sync.dma_start(sel_t[:, :selsplit * P], sel[:, :selsplit * P])
            w1_t = cp.tile([P, KG1 * CH], BF)
            nc.sync.dma_start(w1_t[:], w1[:])
            v1_t = cp.tile([P, KG1 * CH], BF)
            nc.sync.dma_start(v1_t[:], v1[:])
            xT_t = cp.tile([P, KG1 * NPAD], BF)
            nc.sync.dma_start(xT_t[:], xT[:])
            ball_t = cp.tile([P, NLAYERS * CH], F32)
            nc.sync.dma_start(ball_t[:], ball[:])
            nc.sync.dma_start(sel_t[:, selsplit * P:], sel[:, selsplit * P:])
            wk_t = cp.tile([P, 6 * KGC * CH], BF)
            nc.sync.dma_start(wk_t[:], wk[:])
            vk_t = cp.tile([P, 6 * KGC * CH], BF)
            nc.sync.dma_start(vk_t[:], vk[:])
            wd_t = cp.tile([P, KGC * N_LABELS], BF)
            nc.sync.dma_start(wd_t[:], wd[:])
            bdr_t = cp.tile([P, N_LABELS], F32)
            nc.sync.dma_start(bdr_t[:], bdr[:])
            ident = cp.tile([P, P], BF)
            make_identity(nc, ident[:])

            def scatter_phase(l, width, gsrc_ap, tag):
                """Bucket loop: gather tiles from gsrc_ap (DRAM [*,width]),
                scatter-add via sel matmuls, write partial table. Returns
                the partial dram tile."""
                partial = dram.tile([NGPAD, width], BF, tag=tag)
                col = 0
                for b in range(NSB):
                    acc = psagg.tile([P, CH], F32, tag="agg")
                    for k in range(ntiles[b]):
                        msg = msgp.tile([P, CH], BF, tag="msg")
                        nc.gpsimd.indirect_dma_start(
                            out=msg[:, :width], out_offset=None,
                            in_=gsrc_ap,
                            in_offset=bass.IndirectOffsetOnAxis(
                                ap=idx_t[:, col:col + 1], axis=0))
                        nc.tensor.matmul(
                            acc[:, :width],
                            sel_t[:, col * P:(col + 1) * P],
                            msg[:, :width],
                            start=(k == 0), stop=(k == ntiles[b] - 1))
                        col += 1
                    pwt = pwp.tile([P, CH], BF, tag="pw")
                    nc.vector.tensor_copy(pwt[:, :width], acc[:, :width])
                    nc.sync.dma_start(partial[b * P:(b + 1) * P, :],
                                      pwt[:, :width])
                return partial

            for rep in range(repeat):
                # ================= layer 1 (x-form) =================
                # hv1 = x @ V1 + b1, precomputed to SBUF (overlaps scatter/RS)
                hv_sb = []
                for b in range(NBL):
                    hps = psm.tile([P, CH], F32, tag="m")
                    for g in range(KG1):
                        nc.tensor.matmul(
                            hps[:],
                            xT_t[:, g * NPAD + b * P:g * NPAD + (b + 1) * P],
                            v1_t[:, g * CH:(g + 1) * CH],
                            start=(g == 0), stop=(g == KG1 - 1))
                    hvb = hvp.tile([P, CH], BF, tag="hv")
                    nc.vector.tensor_add(hvb[:], hps[:], ball_t[:, 0:CH])
                    hv_sb.append(hvb)

                partial = scatter_phase(0, IN_F, xrows[:], f"ptx{rep}")
                rs1 = dram.tile([NPAD, IN_F], BF, tag=f"rs0_{rep}")
                nc.gpsimd.collective_compute(
                    "ReduceScatter", mybir.AluOpType.add,
                    replica_groups=[list(range(NCORES))],
                    ins=[partial[:].opt()], outs=[rs1[:].opt()])

                hT_cur = htp.tile([P, KGC * NPAD], BF, tag="hT")
                for b in range(NBL):
                    u_sb = aggp.tile([P, CH], BF, tag="aggs")
                    nc.sync.dma_start(u_sb[:, :IN_F], rs1[b * P:(b + 1) * P, :])
                    utt = utp.tile([P, IN_F], BF, tag="ut")
                    for g in range(KG1):
                        tr = pstr.tile([P, P], BF, tag="tr")
                        nc.tensor.transpose(
                            tr[:], u_sb[:, g * P:(g + 1) * P], ident[:])
                        nc.vector.tensor_copy(utt[:, g * P:(g + 1) * P], tr[:])
                    ups = psm.tile([P, CH], F32, tag="m")
                    for g in range(KG1):
                        nc.tensor.matmul(
                            ups[:], utt[:, g * P:(g + 1) * P],
                            w1_t[:, g * CH:(g + 1) * CH],
                            start=(g == 0), stop=(g == KG1 - 1))
                    ep = epp.tile([P, CH], BF, tag="ep")
                    nc.vector.tensor_add(ep[:], ups[:], hv_sb[b][:])
                    h_bf = hp.tile([P, CH], BF, tag="h")
                    nc.scalar.activation(
                        h_bf[:], ep[:], mybir.ActivationFunctionType.Relu)
                    for cg in range(KGC):
                        tr2 = pstr.tile([P, P], BF, tag="tr")
                        nc.tensor.transpose(
                            tr2[:], h_bf[:, cg * P:(cg + 1) * P], ident[:])
                        nc.vector.tensor_copy(
                            hT_cur[:, cg * NPAD + b * P:cg * NPAD + (b + 1) * P],
                            tr2[:])

                # ================= layers 2..7 (m-form) =================
                for l in range(1, NLAYERS):
                    wt = wk_t[:, (l - 1) * KGC * CH:l * KGC * CH]
                    vt = vk_t[:, (l - 1) * KGC * CH:l * KGC * CH]

                    # m = h @ W -> DRAM m-table (gather source)
                    mtab = dram.tile([NPAD, CH], BF, tag=f"mt{l % 2}")
                    for b in range(NBL):
                        mps = psm.tile([P, CH], F32, tag="m")
                        for g in range(KGC):
                            nc.tensor.matmul(
                                mps[:],
                                hT_cur[:, g * NPAD + b * P:g * NPAD + (b + 1) * P],
                                wt[:, g * CH:(g + 1) * CH],
                                start=(g == 0), stop=(g == KGC - 1))
                        m_bf = mp.tile([P, CH], BF, tag="mbf")
                        nc.scalar.activation(
                            m_bf[:], mps[:], mybir.ActivationFunctionType.Copy)
                        nc.sync.dma_start(mtab[b * P:(b + 1) * P, :], m_bf[:])

                    # hv = h @ V + b, precomputed to SBUF
                    hv_sb = []
                    for b in range(NBL):
                        hps = psm.tile([P, CH], F32, tag="m")
                        for g in range(KGC):
                            nc.tensor.matmul(
                                hps[:],
                                hT_cur[:, g * NPAD + b * P:g * NPAD + (b + 1) * P],
                                vt[:, g * CH:(g + 1) * CH],
                                start=(g == 0), stop=(g == KGC - 1))
                        hvb = hvp.tile([P, CH], BF, tag="hv")
                        nc.vector.tensor_add(
                            hvb[:], hps[:], ball_t[:, l * CH:(l + 1) * CH])
                        hv_sb.append(hvb)

                    partial = scatter_phase(l, CH, mtab[:], f"pt{l % 2}_{rep}")
                    rs = dram.tile([NPAD, CH], BF, tag=f"rs{l}_{rep}")
                    nc.gpsimd.collective_compute(
                        "ReduceScatter", mybir.AluOpType.add,
                        replica_groups=[list(range(NCORES))],
                        ins=[partial[:].opt()], outs=[rs[:].opt()])

                    hT_next = htp.tile([P, KGC * NPAD], BF, tag="hT")
                    for b in range(NBL):
                        agg_sb = aggp.tile([P, CH], BF, tag="aggs")
                        nc.sync.dma_start(agg_sb[:], rs[b * P:(b + 1) * P, :])
                        ep = epp.tile([P, CH], BF, tag="ep")
                        nc.vector.tensor_add(ep[:], agg_sb[:], hv_sb[b][:])
                        h_bf = hp.tile([P, CH], BF, tag="h")
                        nc.scalar.activation(
                            h_bf[:], ep[:], mybir.ActivationFunctionType.Relu)
                        for cg in range(KGC):
                            tr2 = pstr.tile([P, P], BF, tag="tr")
                            nc.tensor.transpose(
                                tr2[:], h_bf[:, cg * P:(cg + 1) * P], ident[:])
                            nc.vector.tensor_copy(
                                hT_next[:, cg * NPAD + b * P:cg * NPAD + (b + 1) * P],
                                tr2[:])
                    hT_cur = hT_next

                # ---- final dense: logits = h7 @ Wd + bd
                for b in range(NBL):
                    o_sb = op.tile([P, N_LABELS], F32, tag="o")
                    fps = []
                    for c in range(3):
                        fin_ps = psagg.tile([P, FIN_CHUNK], F32, tag="agg")
                        fps.append(fin_ps)
                    for g in range(KGC):
                        for c in range(3):
                            nc.tensor.matmul(
                                fps[c][:],
                                hT_cur[:, g * NPAD + b * P:g * NPAD + (b + 1) * P],
                                wd_t[:, g * N_LABELS + c * FIN_CHUNK:
                                     g * N_LABELS + (c + 1) * FIN_CHUNK],
                                start=(g == 0), stop=(g == KGC - 1))
                    for c in range(3):
                        sl = slice(c * FIN_CHUNK, (c + 1) * FIN_CHUNK)
                        nc.vector.tensor_add(fps[c][:], fps[c][:], bdr_t[:, sl])
                        nc.scalar.activation(
                            o_sb[:, sl], fps[c][:],
                            mybir.ActivationFunctionType.Copy)
                    if rep == repeat - 1:
                        nc.sync.dma_start(out[b * P:(b + 1) * P, :], o_sb[:])

    _split_excess_waits(nc)
    return nc


# ------------------------------------------------------------- entry point
def kernel(x, src, dst, W1, V1, b1, Wk, Vk, bk, Wd, bd, _repeat=1, _nc_cache={}):
    x = np.asarray(x, np.float32)
    ntiles, idx_tabs, sel_tabs = _prep_edges(src, dst)

    key = (tuple(ntiles), _repeat)
    if key not in _nc_cache:
        _nc_cache[key] = _build(ntiles, repeat=_repeat)
    nc = _nc_cache[key]

    # weights (replicated, host-packed)
    w1p = _pack_rhs(np.asarray(W1, np.float32), KG1, CH).astype(BFNP)
    v1p = _pack_rhs(np.asarray(V1, np.float32), KG1, CH).astype(BFNP)
    wkp = np.concatenate(
        [_pack_rhs(np.asarray(Wk[i], np.float32), KGC, CH) for i in range(6)],
        axis=1).astype(BFNP)
    vkp = np.concatenate(
        [_pack_rhs(np.asarray(Vk[i], np.float32), KGC, CH) for i in range(6)],
        axis=1).astype(BFNP)
    wdp = _pack_rhs(np.asarray(Wd, np.float32), KGC, N_LABELS).astype(BFNP)
    ballv = np.concatenate(
        [np.asarray(b1, np.float32)] + [np.asarray(bk[i], np.float32)
                                        for i in range(6)])
    ballp = np.broadcast_to(ballv, (P, NLAYERS * CH)).copy()
    bdp = np.broadcast_to(np.asarray(bd, np.float32), (P, N_LABELS)).copy()

    in_maps = []
    for p in range(NCORES):
        xp = np.zeros((NPAD, IN_F), np.float32)
        xp[:NPC] = x[p * NPC:(p + 1) * NPC]
        xTp = _pack_lhsT(np.ascontiguousarray(xp.T), KG1).astype(BFNP)
        in_maps.append({
            "xT": xTp, "xrows": xp.astype(BFNP),
            "idx": idx_tabs[p], "sel": sel_tabs[p],
            "w1": w1p, "v1": v1p, "wk": wkp, "vk": vkp, "wd": wdp,
            "ball": ballp, "bdr": bdp,
        })

    res = run_bass_kernel_spmd(nc, in_maps, core_ids=list(range(NCORES)))
    outp = np.empty((N_NODES, N_LABELS), np.float32)
    for p in range(NCORES):
        outp[p * NPC:(p + 1) * NPC] = res.results[p]["out"][:NPC]
    return outp


# revision 12
# speedup vs baseline: 1.1569x; 1.1569x over previous
"""ARMA-style GNN message passing on 8 TRN2 NeuronCores.

Reference computation (per layer, 7 layers):
    m   = h @ W                                  [N, CH]
    agg = segment_sum(w[:,None] * m[dst], src)   [N, CH]
    h'  = relu(agg + h @ V + b)
then logits = h @ Wd + bd.

Strategy (graph/data parallel over nodes, ReduceScatter formulation):
  - 8 cores own 1250 nodes each (padded to 1280 = 10 blocks of 128).
  - Edge (s, d) is processed by the core owning d (where the message row
    m[d] is LOCAL).  Each core computes, per layer, a partial aggregation
    table over the full padded-global src space [8*1280, CH]:
        partial_p[s] = sum_{e: dst_e in p, src_e = s} w_e * m_local[dst_e]
    then one ReduceScatter(add) sums the 8 partials and hands every core
    the [1280, CH] slice for its own nodes.  (A ReduceScatter's output is
    1/8 the size of the AllGather the src-side formulation needs, which is
    what the inter-chip collective cost scales with.)
  - Edges are bucketed by global src block (80 buckets of 128 src slots).
    Within a bucket the distinct local dst rows are deduplicated into
    gather tiles of 128 rows; one indirect DMA fetches a tile from the
    local m table in DRAM, and a [128 dst-rows x 128 src-slots] bf16
    "selection" matrix carrying the degree weights w_e scatter-adds it
    into the bucket's PSUM accumulator on PE.
  - Layer 1 scatters x itself (256 wide, half the traffic) and applies W1
    after the ReduceScatter: A@(x W1) == (A@x) W1.
  - h@V + b is precomputed into SBUF while the ReduceScatter is in
    flight; the post-collective epilogue is add + relu + PE-transpose.
  - Final dense layer and output assembly per core; host concatenates.

All matmuls run in bf16 with fp32 PSUM accumulation.
"""
import numpy as np
import ml_dtypes

import concourse.bass as bass
import concourse.tile as tile
import concourse.mybir as mybir
from concourse.vector_clock import ScopedClock
from concourse.bass_utils import run_bass_kernel_spmd
from concourse.masks import make_identity

# ---------------------------------------------------------------- constants
N_NODES = 10000
N_EDGES = 160000
IN_F = 256
CH = 512
N_LABELS = 1440
NCORES = 8
NPC = N_NODES // NCORES      # 1250 nodes per core
P = 128
NBL = 10                     # node blocks per core (10*128 = 1280)
NPAD = NBL * P               # padded nodes per core
NSB = NCORES * NBL           # global src blocks (80)
NGPAD = NCORES * NPAD        # padded global node space (10240)
NLAYERS = 7
KG1 = IN_F // P              # 2 contraction blocks in layer 1
KGC = CH // P                # 4 contraction blocks in layers 2..7
FIN_CHUNK = 480              # 1440 = 3 * 480, fits one PSUM bank in f32

BF = mybir.dt.bfloat16
F32 = mybir.dt.float32
BFNP = ml_dtypes.bfloat16


# ------------------------------------------------------- walrus workarounds
def _patched_drain_and_barrier(self, tick_clock, wait_clock):
    # This walrus build rejects >1-2 sync waits on one TPB_CTRL; put the
    # kernel-tail drain's waits on separate preceding SP nops instead.
    nc = self.nc
    probe = nc.sync.nop(nofuse=True, hint="drain_waits")
    wait_clock.add_sem_waits(probe.ins, ScopedClock({None: tick_clock.global_clock}))
    si = probe.ins.sync_info
    waits = list(si.on_wait) if si is not None else []
    if len(waits) > 1:
        si.on_wait = waits[:1]
        for i in range(1, len(waits)):
            n2 = nc.sync.nop(nofuse=True, hint=f"drain_waits_{i}")
            n2.ins.sync_info = mybir.SyncInfo(on_wait=[waits[i]], on_update=[])
    nc.sync.drain()
    nc.all_engine_barrier()
    assert self.sems is not None
    popped = nc._tile_sem_poison_stack.pop()
    assert popped is self._sem_poison
    nc.clear_and_free_semaphores(list(self.sems.allocated().values()))
    nc.all_engine_barrier()


tile.TileContext._drain_and_barrier = _patched_drain_and_barrier


def _split_excess_waits(nc, limit=1):
    # Same ISA restriction for ordinary instructions: hoist excess sync
    # waits onto injected same-engine nops placed just before.
    for func in nc.m.functions:
        for bb in func.blocks:
            out = []
            for ins in bb.instructions:
                si = ins.sync_info
                if si is not None and si.on_wait and len(si.on_wait) > limit:
                    waits = list(si.on_wait)
                    excess, keep = waits[:-limit], waits[-limit:]
                    for i in range(0, len(excess), limit):
                        out.append(mybir.InstNoOp(
                            name=f"{ins.name}_xw{i}",
                            engine=ins.engine,
                            ins=[], outs=[],
                            sync_info=mybir.SyncInfo(
                                on_wait=excess[i:i + limit], on_update=[]),
                        ))
                    si.on_wait = keep
                out.append(ins)
            bb.instructions[:] = out


# ------------------------------------------------------------- host prep
GSZ = 5                       # src blocks (buckets) per dedupe group
NGRP = NSB // GSZ             # 16 groups; group g = (core g//2, half g%2)


def _prep_edges(src, dst):
    """Route each edge to the core owning dst.  Group the 80 global src
    blocks into 16 groups of 5 (one half of one core's padded node range
    each, so a group maps 1:1 to a ReduceScatter chunk).  Per group the
    distinct local dst rows are deduplicated into gather tiles of 128 and
    gathered once; each tile then feeds GSZ sel-matmuls (one per bucket).
    Returns (ntiles, idx_tabs, sel_tabs): ntiles[g] = tile count of group
    g (max over cores, same program on all cores); per core an idx table
    [128, NT] of local m-table rows and a sel table [128, NT*GSZ*128] of
    bf16 degree weights, columns ordered (group, tile, bucket-in-group)."""
    src = np.asarray(src).astype(np.int64)
    dst = np.asarray(dst).astype(np.int64)
    deg_out = np.maximum(np.bincount(src, minlength=N_NODES), 1.0).astype(np.float32)
    deg_in = np.maximum(np.bincount(dst, minlength=N_NODES), 1.0).astype(np.float32)
    w = (1.0 / np.sqrt(deg_out[src] * deg_in[dst])).astype(np.float32)

    core = dst // NPC
    ldst = dst - core * NPC
    psrc = (src // NPC) * NPAD + (src % NPC)   # padded-global src row
    grp = psrc // (GSZ * P)
    jsl = psrc - grp * (GSZ * P)               # bucket-in-group * 128 + slot

    order = np.lexsort((ldst, grp, core))
    c_s, g_s = core[order], grp[order]
    ld_s, js_s, w_s = ldst[order], jsl[order], w[order]
    key = c_s * NGRP + g_s
    starts = np.searchsorted(key, np.arange(NCORES * NGRP))
    ends = np.searchsorted(key, np.arange(NCORES * NGRP), side="right")

    uniq_store = {}
    ntiles = np.ones(NGRP, np.int64)
    for p in range(NCORES):
        for g in range(NGRP):
            s0, s1 = starts[p * NGRP + g], ends[p * NGRP + g]
            if s1 > s0:
                uniq, inv = np.unique(ld_s[s0:s1], return_inverse=True)
            else:
                uniq, inv = np.zeros(1, np.int64), np.zeros(0, np.int64)
            uniq_store[p, g] = (uniq, inv, s0, s1)
            ntiles[g] = max(ntiles[g], -(-len(uniq) // P))
    colof = np.zeros(NGRP, np.int64)
    colof[1:] = np.cumsum(ntiles)[:-1]
    nt = int(ntiles.sum())

    idx_tabs, sel_tabs = [], []
    for p in range(NCORES):
        idx_t = np.zeros((P, nt), np.int32)
        sel_t = np.zeros((P, nt * GSZ * P), np.float32)
        for g in range(NGRP):
            uniq, inv, s0, s1 = uniq_store[p, g]
            lanes_u = np.arange(len(uniq))
            idx_t[lanes_u % P, colof[g] + lanes_u // P] = uniq
            if s1 > s0:
                tile_e = inv // P
                lane_e = inv - tile_e * P
                # column block for (group g, tile t, bucket j): (co+t)*GSZ+j
                cb = (colof[g] + tile_e) * GSZ + js_s[s0:s1] // P
                np.add.at(sel_t, (lane_e, cb * P + js_s[s0:s1] % P), w_s[s0:s1])
        idx_tabs.append(idx_t)
        sel_tabs.append(sel_t.astype(BFNP))
    return list(ntiles), idx_tabs, sel_tabs


def _pack_lhsT(xT, kg):
    """[kg*128, NPAD] -> [128, kg*NPAD] (partition-major kg blocks)."""
    return np.ascontiguousarray(
        xT.reshape(kg, P, NPAD).transpose(1, 0, 2).reshape(P, kg * NPAD))


def _pack_rhs(Wm, kg, n):
    """[kg*128, n] -> [128, kg*n]."""
    return np.ascontiguousarray(
        Wm.reshape(kg, P, n).transpose(1, 0, 2).reshape(P, kg * n))


# ------------------------------------------------------------- device build
def _build(ntiles, repeat=1):
    nt = sum(ntiles)
    ntmax = max(ntiles)
    colof = np.zeros(NGRP, np.int64)
    colof[1:] = np.cumsum(ntiles)[:-1]
    HROW = GSZ * P                      # rows per RS chunk per core (640)
    nc = bass.Bass("TRN2", target_bir_lowering=False, debug=False,
                   num_devices=NCORES)

    def din(name, shape, dt):
        return nc.dram_tensor(name, shape, dt, kind="ExternalInput").ap()

    xT = din("xT", [P, KG1 * NPAD], BF)
    xrows = din("xrows", [NPAD, IN_F], BF)
    idx = din("idx", [P, nt], mybir.dt.int32)
    sel = din("sel", [P, nt * GSZ * P], BF)
    w1 = din("w1", [P, KG1 * CH], BF)
    v1 = din("v1", [P, KG1 * CH], BF)
    wk = din("wk", [P, 6 * KGC * CH], BF)
    vk = din("vk", [P, 6 * KGC * CH], BF)
    wd = din("wd", [P, KGC * N_LABELS], BF)
    ball = din("ball", [P, NLAYERS * CH], F32)
    bdr = din("bdr", [P, N_LABELS], F32)
    out = nc.dram_tensor("out", [NPAD, N_LABELS], F32, kind="ExternalOutput").ap()

    with tile.TileContext(nc) as tc:
        with (
            tc.tile_pool(name="const", bufs=1) as cp,
            tc.tile_pool(name="ht", bufs=2) as htp,
            tc.tile_pool(name="mout", bufs=3) as mp,
            tc.tile_pool(name="selg", bufs=3) as selp,
            tc.tile_pool(name="msg", bufs=2 * ntmax) as msgp,
            tc.tile_pool(name="pw", bufs=4) as pwp,
            tc.tile_pool(name="agg", bufs=4) as aggp,
            tc.tile_pool(name="hv", bufs=12) as hvp,
            tc.tile_pool(name="ep", bufs=3) as epp,
            tc.tile_pool(name="ut", bufs=2) as utp,
            tc.tile_pool(name="hact", bufs=2) as hp,
            tc.tile_pool(name="outs", bufs=2) as op,
            tc.tile_pool(name="psm", bufs=2, space="PSUM") as psm,
            tc.tile_pool(name="psagg", bufs=3, space="PSUM") as psagg,
            tc.tile_pool(name="pstr", bufs=2, space="PSUM") as pstr,
            tc.tile_pool(name="dram", bufs=1, space="DRAM") as dram,
        ):
            # ---- constants to SBUF, roughly in first-use order
            idx_t = cp.tile([P, nt], mybir.dt.int32)
            nc.sync.dma_start(idx_t[:], idx[:])
            w1_t = cp.tile([P, KG1 * CH], BF)
            nc.sync.dma_start(w1_t[:], w1[:])
            v1_t = cp.tile([P, KG1 * CH], BF)
            nc.sync.dma_start(v1_t[:], v1[:])
            xT_t = cp.tile([P, KG1 * NPAD], BF)
            nc.sync.dma_start(xT_t[:], xT[:])
            ball_t = cp.tile([P, NLAYERS * CH], F32)
            nc.sync.dma_start(ball_t[:], ball[:])
            wk_t = cp.tile([P, 6 * KGC * CH], BF)
            nc.sync.dma_start(wk_t[:], wk[:])
            vk_t = cp.tile([P, 6 * KGC * CH], BF)
            nc.sync.dma_start(vk_t[:], vk[:])
            wd_t = cp.tile([P, KGC * N_LABELS], BF)
            nc.sync.dma_start(wd_t[:], wd[:])
            bdr_t = cp.tile([P, N_LABELS], F32)
            nc.sync.dma_start(bdr_t[:], bdr[:])
            ident = cp.tile([P, P], BF)
            make_identity(nc, ident[:])

            def scatter_phase(width, gsrc_ap, rs_out, ptag):
                """Grouped scatter: per group stream the sel columns, gather
                the deduped dst-row tiles once, scatter-add into the GSZ
                bucket accumulators, write the partial table; after each
                half (8 groups) launch that ReduceScatter chunk into
                rs_out's row range."""
                for hg in (0, 1):
                    partial = dram.tile([NCORES, HROW, width], BF,
                                        tag=f"{ptag}h{hg}")
                    for cg in range(NCORES):
                        g = cg * 2 + hg
                        co, ntg = int(colof[g]), ntiles[g]
                        selg = selp.tile([P, ntmax * GSZ * P], BF, tag="selg")
                        nc.sync.dma_start(
                            selg[:, :ntg * GSZ * P],
                            sel[:, co * GSZ * P:(co + ntg) * GSZ * P])
                        msgs = []
                        for t in range(ntg):
                            msg = msgp.tile([P, CH], BF, tag="msg")
                            nc.gpsimd.indirect_dma_start(
                                out=msg[:, :width], out_offset=None,
                                in_=gsrc_ap,
                                in_offset=bass.IndirectOffsetOnAxis(
                                    ap=idx_t[:, co + t:co + t + 1], axis=0))
                            msgs.append(msg)
                        for j in range(GSZ):
                            acc = psagg.tile([P, CH], F32, tag="agg")
                            for t in range(ntg):
                                nc.tensor.matmul(
                                    acc[:, :width],
                                    selg[:, (t * GSZ + j) * P:(t * GSZ + j + 1) * P],
                                    msgs[t][:, :width],
                                    start=(t == 0), stop=(t == ntg - 1))
                            pwt = pwp.tile([P, CH], BF, tag="pw")
                            nc.vector.tensor_copy(pwt[:, :width], acc[:, :width])
                            nc.sync.dma_start(
                                partial[cg, j * P:(j + 1) * P, :],
                                pwt[:, :width])
                    nc.gpsimd.collective_compute(
                        "ReduceScatter", mybir.AluOpType.add,
                        replica_groups=[list(range(NCORES))],
                        ins=[partial[:].opt()],
                        outs=[rs_out[hg * HROW:(hg + 1) * HROW, :].opt()])

            for rep in range(repeat):
                # ================= layer 1 (x-form) =================
                # hv1 = x @ V1 + b1, precomputed to SBUF (overlaps scatter/RS)
                hv_sb = []
                for b in range(NBL):
                    hps = psm.tile([P, CH], F32, tag="m")
                    for g in range(KG1):
                        nc.tensor.matmul(
                            hps[:],
                            xT_t[:, g * NPAD + b * P:g * NPAD + (b + 1) * P],
                            v1_t[:, g * CH:(g + 1) * CH],
                            start=(g == 0), stop=(g == KG1 - 1))
                    hvb = hvp.tile([P, CH], BF, tag="hv")
                    nc.vector.tensor_add(hvb[:], hps[:], ball_t[:, 0:CH])
                    hv_sb.append(hvb)

                rs1 = dram.tile([NPAD, IN_F], BF, tag=f"rs0_{rep}")
                scatter_phase(IN_F, xrows[:], rs1, f"ptx{rep}")

                hT_cur = htp.tile([P, KGC * NPAD], BF, tag="hT")
                for b in range(NBL):
                    u_sb = aggp.tile([P, CH], BF, tag="aggs")
                    nc.sync.dma_start(u_sb[:, :IN_F], rs1[b * P:(b + 1) * P, :])
                    utt = utp.tile([P, IN_F], BF, tag="ut")
                    for g in range(KG1):
                        tr = pstr.tile([P, P], BF, tag="tr")
                        nc.tensor.transpose(
                            tr[:], u_sb[:, g * P:(g + 1) * P], ident[:])
                        nc.vector.tensor_copy(utt[:, g * P:(g + 1) * P], tr[:])
                    ups = psm.tile([P, CH], F32, tag="m")
                    for g in range(KG1):
                        nc.tensor.matmul(
                            ups[:], utt[:, g * P:(g + 1) * P],
                            w1_t[:, g * CH:(g + 1) * CH],
                            start=(g == 0), stop=(g == KG1 - 1))
                    ep = epp.tile([P, CH], BF, tag="ep")
                    nc.vector.tensor_add(ep[:], ups[:], hv_sb[b][:])
                    h_bf = hp.tile([P, CH], BF, tag="h")
                    nc.scalar.activation(
                        h_bf[:], ep[:], mybir.ActivationFunctionType.Relu)
                    for cg in range(KGC):
                        tr2 = pstr.tile([P, P], BF, tag="tr")
                        nc.tensor.transpose(
                            tr2[:], h_bf[:, cg * P:(cg + 1) * P], ident[:])
                        nc.vector.tensor_copy(
                            hT_cur[:, cg * NPAD + b * P:cg * NPAD + (b + 1) * P],
                            tr2[:])

                # ================= layers 2..7 (m-form) =================
                for l in range(1, NLAYERS):
                    wt = wk_t[:, (l - 1) * KGC * CH:l * KGC * CH]
                    vt = vk_t[:, (l - 1) * KGC * CH:l * KGC * CH]

                    # m = h @ W -> DRAM m-table (gather source)
                    mtab = dram.tile([NPAD, CH], BF, tag=f"mt{l % 2}")
                    for b in range(NBL):
                        mps = psm.tile([P, CH], F32, tag="m")
                        for g in range(KGC):
                            nc.tensor.matmul(
                                mps[:],
                                hT_cur[:, g * NPAD + b * P:g * NPAD + (b + 1) * P],
                                wt[:, g * CH:(g + 1) * CH],
                                start=(g == 0), stop=(g == KGC - 1))
                        m_bf = mp.tile([P, CH], BF, tag="mbf")
                        nc.scalar.activation(
                            m_bf[:], mps[:], mybir.ActivationFunctionType.Copy)
                        nc.sync.dma_start(mtab[b * P:(b + 1) * P, :], m_bf[:])

                    rs = dram.tile([NPAD, CH], BF, tag=f"rs{l}_{rep}")
                    scatter_phase(CH, mtab[:], rs, f"pt{l % 2}_{rep}")

                    # epilogue: hv then add+relu+transpose per block; hv and
                    # blocks 0-4 overlap the second ReduceScatter chunk
                    hT_next = htp.tile([P, KGC * NPAD], BF, tag="hT")
                    for b in range(NBL):
                        hps = psm.tile([P, CH], F32, tag="m")
                        for g in range(KGC):
                            nc.tensor.matmul(
                                hps[:],
                                hT_cur[:, g * NPAD + b * P:g * NPAD + (b + 1) * P],
                                vt[:, g * CH:(g + 1) * CH],
                                start=(g == 0), stop=(g == KGC - 1))
                        hvb = hvp.tile([P, CH], BF, tag="hv")
                        nc.vector.tensor_add(
                            hvb[:], hps[:], ball_t[:, l * CH:(l + 1) * CH])
                        agg_sb = aggp.tile([P, CH], BF, tag="aggs")
                        nc.sync.dma_start(agg_sb[:], rs[b * P:(b + 1) * P, :])
                        ep = epp.tile([P, CH], BF, tag="ep")
                        nc.vector.tensor_add(ep[:], agg_sb[:], hvb[:])
                        h_bf = hp.tile([P, CH], BF, tag="h")
                        nc.scalar.activation(
                            h_bf[:], ep[:], mybir.ActivationFunctionType.Relu)
                        for cg in range(KGC):
                            tr2 = pstr.tile([P, P], BF, tag="tr")
                            nc.tensor.transpose(
                                tr2[:], h_bf[:, cg * P:(cg + 1) * P], ident[:])
                            nc.vector.tensor_copy(
                                hT_next[:, cg * NPAD + b * P:cg * NPAD + (b + 1) * P],
                                tr2[:])
                    hT_cur = hT_next

                # ---- final dense: logits = h7 @ Wd + bd
                for b in range(NBL):
                    o_sb = op.tile([P, N_LABELS], F32, tag="o")
                    fps = []
                    for c in range(3):
                        fin_ps = psagg.tile([P, FIN_CHUNK], F32, tag="agg")
                        fps.append(fin_ps)
                    for g in range(KGC):
                        for c in range(3):
                            nc.tensor.matmul(
                                fps[c][:],
                                hT_cur[:, g * NPAD + b * P:g * NPAD + (b + 1) * P],
                                wd_t[:, g * N_LABELS + c * FIN_CHUNK:
                                     g * N_LABELS + (c + 1) * FIN_CHUNK],
                                start=(g == 0), stop=(g == KGC - 1))
                    for c in range(3):
                        sl = slice(c * FIN_CHUNK, (c + 1) * FIN_CHUNK)
                        nc.vector.tensor_add(fps[c][:], fps[c][:], bdr_t[:, sl])
                        nc.scalar.activation(
                            o_sb[:, sl], fps[c][:],
                            mybir.ActivationFunctionType.Copy)
                    if rep == repeat - 1:
                        nc.sync.dma_start(out[b * P:(b + 1) * P, :], o_sb[:])

    _split_excess_waits(nc)
    return nc


# ------------------------------------------------------------- entry point
def kernel(x, src, dst, W1, V1, b1, Wk, Vk, bk, Wd, bd, _repeat=1, _nc_cache={}):
    x = np.asarray(x, np.float32)
    ntiles, idx_tabs, sel_tabs = _prep_edges(src, dst)

    key = (tuple(ntiles), _repeat)
    if key not in _nc_cache:
        _nc_cache[key] = _build(ntiles, repeat=_repeat)
    nc = _nc_cache[key]

    # weights (replicated, host-packed)
    w1p = _pack_rhs(np.asarray(W1, np.float32), KG1, CH).astype(BFNP)
    v1p = _pack_rhs(np.asarray(V1, np.float32), KG1, CH).astype(BFNP)
    wkp = np.concatenate(
        [_pack_rhs(np.asarray(Wk[i], np.float32), KGC, CH) for i in range(6)],
        axis=1).astype(BFNP)
    vkp = np.concatenate(
        [_pack_rhs(np.asarray(Vk[i], np.float32), KGC, CH) for i in range(6)],
        axis=1).astype(BFNP)
    wdp = _pack_rhs(np.asarray(Wd, np.float32), KGC, N_LABELS).astype(BFNP)
    ballv = np.concatenate(
        [np.asarray(b1, np.float32)] + [np.asarray(bk[i], np.float32)
                                        for i in range(6)])
    ballp = np.broadcast_to(ballv, (P, NLAYERS * CH)).copy()
    bdp = np.broadcast_to(np.asarray(bd, np.float32), (P, N_LABELS)).copy()

    in_maps = []
    for p in range(NCORES):
        xp = np.zeros((NPAD, IN_F), np.float32)
        xp[:NPC] = x[p * NPC:(p + 1) * NPC]
        xTp = _pack_lhsT(np.ascontiguousarray(xp.T), KG1).astype(BFNP)
        in_maps.append({
            "xT": xTp, "xrows": xp.astype(BFNP),
            "idx": idx_tabs[p], "sel": sel_tabs[p],
            "w1": w1p, "v1": v1p, "wk": wkp, "vk": vkp, "wd": wdp,
            "ball": ballp, "bdr": bdp,
        })

    res = run_bass_kernel_spmd(nc, in_maps, core_ids=list(range(NCORES)))
    outp = np.empty((N_NODES, N_LABELS), np.float32)
    for p in range(NCORES):
        outp[p * NPC:(p + 1) * NPC] = res.results[p]["out"][:NPC]
    return outp


# revision 18
# speedup vs baseline: 1.1876x; 1.0265x over previous
"""ARMA-style GNN message passing on 8 TRN2 NeuronCores.

Reference computation (per layer, 7 layers):
    m   = h @ W                                  [N, CH]
    agg = segment_sum(w[:,None] * m[dst], src)   [N, CH]
    h'  = relu(agg + h @ V + b)
then logits = h @ Wd + bd.

Strategy (graph/data parallel over nodes, ReduceScatter formulation):
  - 8 cores own 1250 nodes each (padded to 1280 = 10 blocks of 128).
  - Edge (s, d) is processed by the core owning d (where the message row
    m[d] is LOCAL).  Each core computes, per layer, a partial aggregation
    table over the full padded-global src space [8*1280, CH]:
        partial_p[s] = sum_{e: dst_e in p, src_e = s} w_e * m_local[dst_e]
    then one ReduceScatter(add) sums the 8 partials and hands every core
    the [1280, CH] slice for its own nodes.  (A ReduceScatter's output is
    1/8 the size of the AllGather the src-side formulation needs, which is
    what the inter-chip collective cost scales with.)
  - Edges are bucketed by global src block (80 buckets of 128 src slots).
    Within a bucket the distinct local dst rows are deduplicated into
    gather tiles of 128 rows; one indirect DMA fetches a tile from the
    local m table in DRAM, and a [128 dst-rows x 128 src-slots] bf16
    "selection" matrix carrying the degree weights w_e scatter-adds it
    into the bucket's PSUM accumulator on PE.
  - Layer 1 scatters x itself (256 wide, half the traffic) and applies W1
    after the ReduceScatter: A@(x W1) == (A@x) W1.
  - h@V + b is precomputed into SBUF while the ReduceScatter is in
    flight; the post-collective epilogue is add + relu + PE-transpose.
  - Final dense layer and output assembly per core; host concatenates.

All matmuls run in bf16 with fp32 PSUM accumulation.
"""
import numpy as np
import ml_dtypes

import concourse.bass as bass
import concourse.tile as tile
import concourse.mybir as mybir
from concourse.vector_clock import ScopedClock
from concourse.bass_utils import run_bass_kernel_spmd
from concourse.masks import make_identity

# ---------------------------------------------------------------- constants
N_NODES = 10000
N_EDGES = 160000
IN_F = 256
CH = 512
N_LABELS = 1440
NCORES = 8
NPC = N_NODES // NCORES      # 1250 nodes per core
P = 128
NBL = 10                     # node blocks per core (10*128 = 1280)
NPAD = NBL * P               # padded nodes per core
NSB = NCORES * NBL           # global src blocks (80)
NGPAD = NCORES * NPAD        # padded global node space (10240)
NLAYERS = 7
KG1 = IN_F // P              # 2 contraction blocks in layer 1
KGC = CH // P                # 4 contraction blocks in layers 2..7
FIN_CHUNK = 480              # 1440 = 3 * 480, fits one PSUM bank in f32

BF = mybir.dt.bfloat16
F32 = mybir.dt.float32
BFNP = ml_dtypes.bfloat16


# ------------------------------------------------------- walrus workarounds
def _patched_drain_and_barrier(self, tick_clock, wait_clock):
    # This walrus build rejects >1-2 sync waits on one TPB_CTRL; put the
    # kernel-tail drain's waits on separate preceding SP nops instead.
    nc = self.nc
    probe = nc.sync.nop(nofuse=True, hint="drain_waits")
    wait_clock.add_sem_waits(probe.ins, ScopedClock({None: tick_clock.global_clock}))
    si = probe.ins.sync_info
    waits = list(si.on_wait) if si is not None else []
    if len(waits) > 1:
        si.on_wait = waits[:1]
        for i in range(1, len(waits)):
            n2 = nc.sync.nop(nofuse=True, hint=f"drain_waits_{i}")
            n2.ins.sync_info = mybir.SyncInfo(on_wait=[waits[i]], on_update=[])
    nc.sync.drain()
    nc.all_engine_barrier()
    assert self.sems is not None
    popped = nc._tile_sem_poison_stack.pop()
    assert popped is self._sem_poison
    nc.clear_and_free_semaphores(list(self.sems.allocated().values()))
    nc.all_engine_barrier()


tile.TileContext._drain_and_barrier = _patched_drain_and_barrier


def _split_excess_waits(nc, limit=1):
    # Same ISA restriction for ordinary instructions: hoist excess sync
    # waits onto injected same-engine nops placed just before.
    for func in nc.m.functions:
        for bb in func.blocks:
            out = []
            for ins in bb.instructions:
                si = ins.sync_info
                if si is not None and si.on_wait and len(si.on_wait) > limit:
                    waits = list(si.on_wait)
                    excess, keep = waits[:-limit], waits[-limit:]
                    for i in range(0, len(excess), limit):
                        out.append(mybir.InstNoOp(
                            name=f"{ins.name}_xw{i}",
                            engine=ins.engine,
                            ins=[], outs=[],
                            sync_info=mybir.SyncInfo(
                                on_wait=excess[i:i + limit], on_update=[]),
                        ))
                    si.on_wait = keep
                out.append(ins)
            bb.instructions[:] = out


# ------------------------------------------------------------- host prep
SUBGROUPS = ((0, 1, 2, 3), (4, 5, 6), (7, 8, 9))   # local src blocks per group
NSG = len(SUBGROUPS)
NGRP = NCORES * NSG           # 24 groups; group g = (core g//NSG, sub g%NSG)
CHUNKS = ((0, 1), (2,))       # subgroup ids per ReduceScatter chunk
CHUNK_R0 = (0, 7)             # first local block of each chunk
CHUNK_NB = (7, 3)             # local blocks per chunk
SUBSZ = tuple(len(s) for s in SUBGROUPS)
SUB_OF = [si for si, s in enumerate(SUBGROUPS) for _ in s]
FIRSTB = tuple(s[0] for s in SUBGROUPS)
SMAX = max(SUBSZ)


def _prep_edges(src, dst):
    """Route each edge to the core owning dst.  The 80 global src blocks
    form 24 groups (per src-owning core: blocks 0-3, 4-6, 7-9; the 7/3
    block split matches the two ReduceScatter chunks).  Per group the
    distinct local dst rows are deduplicated into gather tiles of 128 and
    gathered once; each tile then feeds one sel-matmul per block in the
    group.  Returns (ntiles, idx_tabs, sel_tabs): ntiles[g] = tile count
    of group g (max over cores, same program on all cores); per core an
    idx table [128, NT] of local m-table rows and a sel table of bf16
    degree weights, column blocks ordered (group, tile, block-in-group)."""
    src = np.asarray(src).astype(np.int64)
    dst = np.asarray(dst).astype(np.int64)
    deg_out = np.maximum(np.bincount(src, minlength=N_NODES), 1.0).astype(np.float32)
    deg_in = np.maximum(np.bincount(dst, minlength=N_NODES), 1.0).astype(np.float32)
    w = (1.0 / np.sqrt(deg_out[src] * deg_in[dst])).astype(np.float32)

    core = dst // NPC
    ldst = dst - core * NPC
    psrc = (src // NPC) * NPAD + (src % NPC)   # padded-global src row
    sb = psrc // P
    cg_e = sb // NBL
    lb_e = sb - cg_e * NBL
    si_e = np.asarray(SUB_OF)[lb_e]
    grp = cg_e * NSG + si_e
    j_e = lb_e - np.asarray(FIRSTB)[si_e]      # block index within group
    slot = psrc - sb * P

    order = np.lexsort((ldst, grp, core))
    c_s, g_s = core[order], grp[order]
    ld_s, j_s, sl_s, w_s = ldst[order], j_e[order], slot[order], w[order]
    key = c_s * NGRP + g_s
    starts = np.searchsorted(key, np.arange(NCORES * NGRP))
    ends = np.searchsorted(key, np.arange(NCORES * NGRP), side="right")

    # Gather tiles are split at local dst row EROW: "early" rows live in
    # m-table A (node blocks 0..6, complete while the trailing collective
    # still runs) and "late" rows in m-table B, so most of the next layer's
    # gathers can start under the previous layer's collective tail.
    uniq_store = {}
    ntA = np.zeros(NGRP, np.int64)
    ntB = np.zeros(NGRP, np.int64)
    for p in range(NCORES):
        for g in range(NGRP):
            s0, s1 = starts[p * NGRP + g], ends[p * NGRP + g]
            if s1 > s0:
                uniq, inv = np.unique(ld_s[s0:s1], return_inverse=True)
            else:
                uniq, inv = np.zeros(1, np.int64), np.zeros(0, np.int64)
            na = int((uniq < EROW).sum())
            uniq_store[p, g] = (uniq, inv, na, s0, s1)
            ntA[g] = max(ntA[g], -(-na // P))
            ntB[g] = max(ntB[g], -(-(len(uniq) - na) // P))
    ntA = np.maximum(ntA, 1)
    ntiles = ntA + ntB
    subsz = np.asarray([SUBSZ[g % NSG] for g in range(NGRP)])
    colof = np.zeros(NGRP, np.int64)           # tile-column offsets (idx)
    colof[1:] = np.cumsum(ntiles)[:-1]
    czof = np.zeros(NGRP, np.int64)            # 128-col-block offsets (sel)
    czof[1:] = np.cumsum(ntiles * subsz)[:-1]
    nt = int(ntiles.sum())
    ncb = int((ntiles * subsz).sum())

    idx_tabs, sel_tabs = [], []
    for p in range(NCORES):
        idx_t = np.zeros((P, nt), np.int32)
        sel_t = np.zeros((P, ncb * P), np.float32)
        for g in range(NGRP):
            uniq, inv, na, s0, s1 = uniq_store[p, g]
            nb = len(uniq) - na
            # lanes: A rows tile 0..ntA-1 (by A position), B rows tile
            # ntA.. (by B position, row index rebased to table B)
            la = np.arange(na)
            idx_t[la % P, colof[g] + la // P] = uniq[:na]
            lb = np.arange(nb)
            idx_t[lb % P, colof[g] + int(ntA[g]) + lb // P] = uniq[na:] - EROW
            if s1 > s0:
                pos = inv
                tile_e = np.where(pos < na, pos // P,
                                  ntA[g] + (pos - na) // P)
                lane_e = np.where(pos < na, pos % P, (pos - na) % P)
                cb = czof[g] + tile_e * subsz[g] + j_s[s0:s1]
                np.add.at(sel_t, (lane_e, cb * P + sl_s[s0:s1]), w_s[s0:s1])
        idx_tabs.append(idx_t)
        sel_tabs.append(sel_t.astype(BFNP))
    return [(int(a), int(b)) for a, b in zip(ntA, ntB)], idx_tabs, sel_tabs


def _pack_lhsT(xT, kg):
    """[kg*128, NPAD] -> [128, kg*NPAD] (partition-major kg blocks)."""
    return np.ascontiguousarray(
        xT.reshape(kg, P, NPAD).transpose(1, 0, 2).reshape(P, kg * NPAD))


def _pack_rhs(Wm, kg, n):
    """[kg*128, n] -> [128, kg*n]."""
    return np.ascontiguousarray(
        Wm.reshape(kg, P, n).transpose(1, 0, 2).reshape(P, kg * n))


# ------------------------------------------------------------- device build
def _build(ntiles, repeat=1):
    nt = sum(ntiles)
    ntmax = max(ntiles)
    ntiles_a = np.asarray(ntiles)
    subsz = np.asarray([SUBSZ[g % NSG] for g in range(NGRP)])
    colof = np.zeros(NGRP, np.int64)
    colof[1:] = np.cumsum(ntiles_a)[:-1]
    czof = np.zeros(NGRP, np.int64)
    czof[1:] = np.cumsum(ntiles_a * subsz)[:-1]
    ncb = int((ntiles_a * subsz).sum())
    nc = bass.Bass("TRN2", target_bir_lowering=False, debug=False,
                   num_devices=NCORES)

    def din(name, shape, dt):
        return nc.dram_tensor(name, shape, dt, kind="ExternalInput").ap()

    xT = din("xT", [P, KG1 * NPAD], BF)
    xrows = din("xrows", [NPAD, IN_F], BF)
    idx = din("idx", [P, nt], mybir.dt.int32)
    sel = din("sel", [P, ncb * P], BF)
    w1 = din("w1", [P, KG1 * CH], BF)
    v1 = din("v1", [P, KG1 * CH], BF)
    wk = din("wk", [P, 6 * KGC * CH], BF)
    vk = din("vk", [P, 6 * KGC * CH], BF)
    wd = din("wd", [P, KGC * N_LABELS], BF)
    ball = din("ball", [P, NLAYERS * CH], F32)
    bdr = din("bdr", [P, N_LABELS], F32)
    out = nc.dram_tensor("out", [NPAD, N_LABELS], F32, kind="ExternalOutput").ap()

    with tile.TileContext(nc) as tc:
        with (
            tc.tile_pool(name="const", bufs=1) as cp,
            tc.tile_pool(name="ht", bufs=2) as htp,
            tc.tile_pool(name="mout", bufs=3) as mp,
            tc.tile_pool(name="selg", bufs=3) as selp,
            tc.tile_pool(name="msg", bufs=2 * ntmax) as msgp,
            tc.tile_pool(name="pw", bufs=4) as pwp,
            tc.tile_pool(name="agg", bufs=4) as aggp,
            tc.tile_pool(name="hv", bufs=12) as hvp,
            tc.tile_pool(name="ep", bufs=3) as epp,
            tc.tile_pool(name="ut", bufs=2) as utp,
            tc.tile_pool(name="hact", bufs=2) as hp,
            tc.tile_pool(name="outs", bufs=2) as op,
            tc.tile_pool(name="psm", bufs=2, space="PSUM") as psm,
            tc.tile_pool(name="psagg", bufs=3, space="PSUM") as psagg,
            tc.tile_pool(name="pstr", bufs=2, space="PSUM") as pstr,
            tc.tile_pool(name="dram", bufs=1, space="DRAM") as dram,
        ):
            # ---- constants to SBUF, roughly in first-use order
            idx_t = cp.tile([P, nt], mybir.dt.int32)
            nc.sync.dma_start(idx_t[:], idx[:])
            w1_t = cp.tile([P, KG1 * CH], BF)
            nc.sync.dma_start(w1_t[:], w1[:])
            v1_t = cp.tile([P, KG1 * CH], BF)
            nc.sync.dma_start(v1_t[:], v1[:])
            xT_t = cp.tile([P, KG1 * NPAD], BF)
            nc.sync.dma_start(xT_t[:], xT[:])
            ball_t = cp.tile([P, NLAYERS * CH], F32)
            nc.sync.dma_start(ball_t[:], ball[:])
            wk_t = cp.tile([P, 6 * KGC * CH], BF)
            nc.sync.dma_start(wk_t[:], wk[:])
            vk_t = cp.tile([P, 6 * KGC * CH], BF)
            nc.sync.dma_start(vk_t[:], vk[:])
            wd_t = cp.tile([P, KGC * N_LABELS], BF)
            nc.sync.dma_start(wd_t[:], wd[:])
            bdr_t = cp.tile([P, N_LABELS], F32)
            nc.sync.dma_start(bdr_t[:], bdr[:])
            ident = cp.tile([P, P], BF)
            make_identity(nc, ident[:])

            def scatter_phase(width, gsrc_ap, rs_out, ptag):
                """Grouped scatter: per group stream the sel columns (on the
                Activation HWDGE queue so they prefetch under the previous
                collective), gather the deduped dst-row tiles once,
                scatter-add into each block's accumulator, write the partial
                table; after each chunk's groups launch that ReduceScatter
                chunk into rs_out's row range."""
                for ci, subids in enumerate(CHUNKS):
                    partial = dram.tile([NCORES, CHUNK_NB[ci] * P, width], BF,
                                        tag=f"{ptag}c{ci}")
                    for cg in range(NCORES):
                        for si in subids:
                            g = cg * NSG + si
                            co, cz = int(colof[g]), int(czof[g])
                            ntg, S = ntiles[g], SUBSZ[si]
                            selg = selp.tile([P, ntmax * SMAX * P], BF,
                                             tag="selg")
                            nc.scalar.dma_start(
                                selg[:, :ntg * S * P],
                                sel[:, cz * P:(cz + ntg * S) * P])
                            msgs = []
                            for t in range(ntg):
                                msg = msgp.tile([P, CH], BF, tag="msg")
                                nc.gpsimd.indirect_dma_start(
                                    out=msg[:, :width], out_offset=None,
                                    in_=gsrc_ap,
                                    in_offset=bass.IndirectOffsetOnAxis(
                                        ap=idx_t[:, co + t:co + t + 1], axis=0))
                                msgs.append(msg)
                            row0 = (FIRSTB[si] - CHUNK_R0[ci]) * P
                            for j in range(S):
                                acc = psagg.tile([P, CH], F32, tag="agg")
                                for t in range(ntg):
                                    nc.tensor.matmul(
                                        acc[:, :width],
                                        selg[:, (t * S + j) * P:(t * S + j + 1) * P],
                                        msgs[t][:, :width],
                                        start=(t == 0), stop=(t == ntg - 1))
                                pwt = pwp.tile([P, CH], BF, tag="pw")
                                nc.vector.tensor_copy(pwt[:, :width],
                                                      acc[:, :width])
                                nc.sync.dma_start(
                                    partial[cg, row0 + j * P:
                                            row0 + (j + 1) * P, :],
                                    pwt[:, :width])
                    nc.gpsimd.collective_compute(
                        "ReduceScatter", mybir.AluOpType.add,
                        replica_groups=[list(range(NCORES))],
                        ins=[partial[:].opt()],
                        outs=[rs_out[CHUNK_R0[ci] * P:
                                     (CHUNK_R0[ci] + CHUNK_NB[ci]) * P,
                                     :].opt()])

            for rep in range(repeat):
                # ================= layer 1 (x-form) =================
                # hv1 = x @ V1 + b1, precomputed to SBUF (overlaps scatter/RS)
                hv_sb = []
                for b in range(NBL):
                    hps = psm.tile([P, CH], F32, tag="m")
                    for g in range(KG1):
                        nc.tensor.matmul(
                            hps[:],
                            xT_t[:, g * NPAD + b * P:g * NPAD + (b + 1) * P],
                            v1_t[:, g * CH:(g + 1) * CH],
                            start=(g == 0), stop=(g == KG1 - 1))
                    hvb = hvp.tile([P, CH], BF, tag="hv")
                    nc.vector.tensor_add(hvb[:], hps[:], ball_t[:, 0:CH])
                    hv_sb.append(hvb)

                rs1 = dram.tile([NPAD, IN_F], BF, tag=f"rs0_{rep}")
                scatter_phase(IN_F, xrows[:], rs1, f"ptx{rep}")

                def emit_m(hT, b, l_next, mtab_next):
                    """m = h @ W for node block b of the next conv layer,
                    written into its m-table (folded into the epilogue so
                    blocks overlap the trailing ReduceScatter chunk)."""
                    wt = wk_t[:, (l_next - 1) * KGC * CH:l_next * KGC * CH]
                    mps = psm.tile([P, CH], F32, tag="m")
                    for g in range(KGC):
                        nc.tensor.matmul(
                            mps[:],
                            hT[:, g * NPAD + b * P:g * NPAD + (b + 1) * P],
                            wt[:, g * CH:(g + 1) * CH],
                            start=(g == 0), stop=(g == KGC - 1))
                    m_bf = mp.tile([P, CH], BF, tag="mbf")
                    nc.scalar.activation(
                        m_bf[:], mps[:], mybir.ActivationFunctionType.Copy)
                    nc.sync.dma_start(mtab_next[b * P:(b + 1) * P, :], m_bf[:])

                def emit_final(hT, b):
                    o_sb = op.tile([P, N_LABELS], F32, tag="o")
                    fps = []
                    for c in range(3):
                        fin_ps = psagg.tile([P, FIN_CHUNK], F32, tag="agg")
                        fps.append(fin_ps)
                    for g in range(KGC):
                        for c in range(3):
                            nc.tensor.matmul(
                                fps[c][:],
                                hT[:, g * NPAD + b * P:g * NPAD + (b + 1) * P],
                                wd_t[:, g * N_LABELS + c * FIN_CHUNK:
                                     g * N_LABELS + (c + 1) * FIN_CHUNK],
                                start=(g == 0), stop=(g == KGC - 1))
                    for c in range(3):
                        sl = slice(c * FIN_CHUNK, (c + 1) * FIN_CHUNK)
                        nc.vector.tensor_add(fps[c][:], fps[c][:], bdr_t[:, sl])
                        nc.scalar.activation(
                            o_sb[:, sl], fps[c][:],
                            mybir.ActivationFunctionType.Copy)
                    if rep == repeat - 1:
                        nc.sync.dma_start(out[b * P:(b + 1) * P, :], o_sb[:])

                hT_cur = htp.tile([P, KGC * NPAD], BF, tag="hT")
                mtab_next = dram.tile([NPAD, CH], BF, tag="mt1")
                for b in range(NBL):
                    u_sb = aggp.tile([P, CH], BF, tag="aggs")
                    nc.sync.dma_start(u_sb[:, :IN_F], rs1[b * P:(b + 1) * P, :])
                    utt = utp.tile([P, IN_F], BF, tag="ut")
                    for g in range(KG1):
                        tr = pstr.tile([P, P], BF, tag="tr")
                        nc.tensor.transpose(
                            tr[:], u_sb[:, g * P:(g + 1) * P], ident[:])
                        nc.vector.tensor_copy(utt[:, g * P:(g + 1) * P], tr[:])
                    ups = psm.tile([P, CH], F32, tag="m")
                    for g in range(KG1):
                        nc.tensor.matmul(
                            ups[:], utt[:, g * P:(g + 1) * P],
                            w1_t[:, g * CH:(g + 1) * CH],
                            start=(g == 0), stop=(g == KG1 - 1))
                    ep = epp.tile([P, CH], BF, tag="ep")
                    nc.vector.tensor_add(ep[:], ups[:], hv_sb[b][:])
                    h_bf = hp.tile([P, CH], BF, tag="h")
                    nc.scalar.activation(
                        h_bf[:], ep[:], mybir.ActivationFunctionType.Relu)
                    for cg in range(KGC):
                        tr2 = pstr.tile([P, P], BF, tag="tr")
                        nc.tensor.transpose(
                            tr2[:], h_bf[:, cg * P:(cg + 1) * P], ident[:])
                        nc.vector.tensor_copy(
                            hT_cur[:, cg * NPAD + b * P:cg * NPAD + (b + 1) * P],
                            tr2[:])
                    emit_m(hT_cur, b, 1, mtab_next)

                # ================= layers 2..7 (m-form) =================
                for l in range(1, NLAYERS):
                    vt = vk_t[:, (l - 1) * KGC * CH:l * KGC * CH]
                    mtab = mtab_next

                    rs = dram.tile([NPAD, CH], BF, tag=f"rs{l}_{rep}")
                    scatter_phase(CH, mtab[:], rs, f"pt{l % 2}_{rep}")

                    # epilogue: per block hv, add+relu, transposes, then the
                    # next layer's m (or the final dense) for that block;
                    # blocks 0-4 overlap the second ReduceScatter chunk
                    if l < NLAYERS - 1:
                        mtab_next = dram.tile([NPAD, CH], BF,
                                              tag=f"mt{(l + 1) % 2}")
                    hT_next = htp.tile([P, KGC * NPAD], BF, tag="hT")
                    for b in range(NBL):
                        hps = psm.tile([P, CH], F32, tag="m")
                        for g in range(KGC):
                            nc.tensor.matmul(
                                hps[:],
                                hT_cur[:, g * NPAD + b * P:g * NPAD + (b + 1) * P],
                                vt[:, g * CH:(g + 1) * CH],
                                start=(g == 0), stop=(g == KGC - 1))
                        hvb = hvp.tile([P, CH], BF, tag="hv")
                        nc.vector.tensor_add(
                            hvb[:], hps[:], ball_t[:, l * CH:(l + 1) * CH])
                        agg_sb = aggp.tile([P, CH], BF, tag="aggs")
                        nc.sync.dma_start(agg_sb[:], rs[b * P:(b + 1) * P, :])
                        ep = epp.tile([P, CH], BF, tag="ep")
                        nc.vector.tensor_add(ep[:], agg_sb[:], hvb[:])
                        h_bf = hp.tile([P, CH], BF, tag="h")
                        nc.scalar.activation(
                            h_bf[:], ep[:], mybir.ActivationFunctionType.Relu)
                        for cg in range(KGC):
                            tr2 = pstr.tile([P, P], BF, tag="tr")
                            nc.tensor.transpose(
                                tr2[:], h_bf[:, cg * P:(cg + 1) * P], ident[:])
                            nc.vector.tensor_copy(
                                hT_next[:, cg * NPAD + b * P:cg * NPAD + (b + 1) * P],
                                tr2[:])
                        if l < NLAYERS - 1:
                            emit_m(hT_next, b, l + 1, mtab_next)
                        else:
                            emit_final(hT_next, b)
                    hT_cur = hT_next

    _split_excess_waits(nc)
    return nc


# ------------------------------------------------------------- entry point
def kernel(x, src, dst, W1, V1, b1, Wk, Vk, bk, Wd, bd, _repeat=1, _nc_cache={}):
    x = np.asarray(x, np.float32)
    ntiles, idx_tabs, sel_tabs = _prep_edges(src, dst)

    key = (tuple(ntiles), _repeat)
    if key not in _nc_cache:
        _nc_cache[key] = _build(ntiles, repeat=_repeat)
    nc = _nc_cache[key]

    # weights (replicated, host-packed)
    w1p = _pack_rhs(np.asarray(W1, np.float32), KG1, CH).astype(BFNP)
    v1p = _pack_rhs(np.asarray(V1, np.float32), KG1, CH).astype(BFNP)
    wkp = np.concatenate(
        [_pack_rhs(np.asarray(Wk[i], np.float32), KGC, CH) for i in range(6)],
        axis=1).astype(BFNP)
    vkp = np.concatenate(
        [_pack_rhs(np.asarray(Vk[i], np.float32), KGC, CH) for i in range(6)],
        axis=1).astype(BFNP)
    wdp = _pack_rhs(np.asarray(Wd, np.float32), KGC, N_LABELS).astype(BFNP)
    ballv = np.concatenate(
        [np.asarray(b1, np.float32)] + [np.asarray(bk[i], np.float32)
                                        for i in range(6)])
    ballp = np.broadcast_to(ballv, (P, NLAYERS * CH)).copy()
    bdp = np.broadcast_to(np.asarray(bd, np.float32), (P, N_LABELS)).copy()

    in_maps = []
    for p in range(NCORES):
        xp = np.zeros((NPAD, IN_F), np.float32)
        xp[:NPC] = x[p * NPC:(p + 1) * NPC]
        xTp = _pack_lhsT(np.ascontiguousarray(xp.T), KG1).astype(BFNP)
        in_maps.append({
            "xT": xTp, "xrows": xp.astype(BFNP),
            "idx": idx_tabs[p], "sel": sel_tabs[p],
            "w1": w1p, "v1": v1p, "wk": wkp, "vk": vkp, "wd": wdp,
            "ball": ballp, "bdr": bdp,
        })

    res = run_bass_kernel_spmd(nc, in_maps, core_ids=list(range(NCORES)))
    outp = np.empty((N_NODES, N_LABELS), np.float32)
    for p in range(NCORES):
        outp[p * NPC:(p + 1) * NPC] = res.results[p]["out"][:NPC]
    return outp


# revision 58
# speedup vs baseline: 1.2200x; 1.0273x over previous
"""ARMA-style GNN message passing on 8 TRN2 NeuronCores.

Reference computation (per layer, 7 layers):
    m   = h @ W                                  [N, CH]
    agg = segment_sum(w[:,None] * m[dst], src)   [N, CH]
    h'  = relu(agg + h @ V + b)
then logits = h @ Wd + bd.

Strategy (graph/data parallel over nodes, ReduceScatter formulation):
  - 8 cores own 1250 nodes each (padded to 1280 = 10 blocks of 128).
  - Edge (s, d) is processed by the core owning d (where the message row
    m[d] is LOCAL).  Each core computes, per layer, a partial aggregation
    table over the full padded-global src space [8*1280, CH]:
        partial_p[s] = sum_{e: dst_e in p, src_e = s} w_e * m_local[dst_e]
    then one ReduceScatter(add) sums the 8 partials and hands every core
    the [1280, CH] slice for its own nodes.  (A ReduceScatter's output is
    1/8 the size of the AllGather the src-side formulation needs, which is
    what the inter-chip collective cost scales with.)
  - Edges are bucketed by global src block (80 buckets of 128 src slots).
    Within a bucket the distinct local dst rows are deduplicated into
    gather tiles of 128 rows; one indirect DMA fetches a tile from the
    local m table in DRAM, and a [128 dst-rows x 128 src-slots] bf16
    "selection" matrix carrying the degree weights w_e scatter-adds it
    into the bucket's PSUM accumulator on PE.
  - Layer 1 scatters x itself (256 wide, half the traffic) and applies W1
    after the ReduceScatter: A@(x W1) == (A@x) W1.
  - h@V + b is precomputed into SBUF while the ReduceScatter is in
    flight; the post-collective epilogue is add + relu + PE-transpose.
  - Final dense layer and output assembly per core; host concatenates.

All matmuls run in bf16 with fp32 PSUM accumulation.
"""
import numpy as np
import ml_dtypes

import concourse.bass as bass
import concourse.tile as tile
import concourse.mybir as mybir
from concourse.vector_clock import ScopedClock
from concourse.bass_utils import run_bass_kernel_spmd
from concourse.masks import make_identity

# ---------------------------------------------------------------- constants
N_NODES = 10000
N_EDGES = 160000
IN_F = 256
CH = 512
N_LABELS = 1440
NCORES = 8
NPC = N_NODES // NCORES      # 1250 nodes per core
P = 128
NBL = 10                     # node blocks per core (10*128 = 1280)
NPAD = NBL * P               # padded nodes per core
NSB = NCORES * NBL           # global src blocks (80)
NGPAD = NCORES * NPAD        # padded global node space (10240)
NLAYERS = 7
KG1 = IN_F // P              # 2 contraction blocks in layer 1
KGC = CH // P                # 4 contraction blocks in layers 2..7
FIN_CHUNK = 480              # 1440 = 3 * 480, fits one PSUM bank in f32

BF = mybir.dt.bfloat16
F32 = mybir.dt.float32
BFNP = ml_dtypes.bfloat16


# ------------------------------------------------------- walrus workarounds
def _patched_drain_and_barrier(self, tick_clock, wait_clock):
    # This walrus build rejects >1-2 sync waits on one TPB_CTRL; put the
    # kernel-tail drain's waits on separate preceding SP nops instead.
    nc = self.nc
    probe = nc.sync.nop(nofuse=True, hint="drain_waits")
    wait_clock.add_sem_waits(probe.ins, ScopedClock({None: tick_clock.global_clock}))
    si = probe.ins.sync_info
    waits = list(si.on_wait) if si is not None else []
    if len(waits) > 1:
        si.on_wait = waits[:1]
        for i in range(1, len(waits)):
            n2 = nc.sync.nop(nofuse=True, hint=f"drain_waits_{i}")
            n2.ins.sync_info = mybir.SyncInfo(on_wait=[waits[i]], on_update=[])
    nc.sync.drain()
    nc.all_engine_barrier()
    assert self.sems is not None
    popped = nc._tile_sem_poison_stack.pop()
    assert popped is self._sem_poison
    nc.clear_and_free_semaphores(list(self.sems.allocated().values()))
    nc.all_engine_barrier()


tile.TileContext._drain_and_barrier = _patched_drain_and_barrier


def _split_excess_waits(nc, limit=1):
    # Same ISA restriction for ordinary instructions: hoist excess sync
    # waits onto injected same-engine nops placed just before.
    for func in nc.m.functions:
        for bb in func.blocks:
            out = []
            for ins in bb.instructions:
                si = ins.sync_info
                if si is not None and si.on_wait and len(si.on_wait) > limit:
                    waits = list(si.on_wait)
                    excess, keep = waits[:-limit], waits[-limit:]
                    for i in range(0, len(excess), limit):
                        out.append(mybir.InstNoOp(
                            name=f"{ins.name}_xw{i}",
                            engine=ins.engine,
                            ins=[], outs=[],
                            sync_info=mybir.SyncInfo(
                                on_wait=excess[i:i + limit], on_update=[]),
                        ))
                    si.on_wait = keep
                out.append(ins)
            bb.instructions[:] = out


# ------------------------------------------------------------- host prep
SUBGROUPS = ((0, 1, 2, 3), (4, 5, 6), (7, 8, 9))   # local src blocks per group
NSG = len(SUBGROUPS)
NGRP = NCORES * NSG           # 24 groups; group g = (core g//NSG, sub g%NSG)
CHUNKS = ((0, 1), (2,))       # subgroup ids per ReduceScatter chunk
CHUNK_R0 = (0, 7)             # first local block of each chunk
CHUNK_NB = (7, 3)             # local blocks per chunk
SUBSZ = tuple(len(s) for s in SUBGROUPS)
SUB_OF = [si for si, s in enumerate(SUBGROUPS) for _ in s]
FIRSTB = tuple(s[0] for s in SUBGROUPS)
SMAX = max(SUBSZ)
EROW = CHUNK_NB[0] * P        # gather rows below this are written early
MID_BACK = 2                  # epilogue insert: groups before chunk-1 end


def _prep_edges(src, dst):
    """Route each edge to the core owning dst.  The 80 global src blocks
    form 24 groups (per src-owning core: blocks 0-3, 4-6, 7-9; the 7/3
    block split matches the two ReduceScatter chunks).  Per group the
    distinct local dst rows are deduplicated into gather tiles of 128 and
    gathered once; each tile then feeds one sel-matmul per block in the
    group.  Returns (ntiles, idx_tabs, sel_tabs): ntiles[g] = tile count
    of group g (max over cores, same program on all cores); per core an
    idx table [128, NT] of local m-table rows and a sel table of bf16
    degree weights, column blocks ordered (group, tile, block-in-group)."""
    src = np.asarray(src).astype(np.int64)
    dst = np.asarray(dst).astype(np.int64)
    deg_out = np.maximum(np.bincount(src, minlength=N_NODES), 1.0).astype(np.float32)
    deg_in = np.maximum(np.bincount(dst, minlength=N_NODES), 1.0).astype(np.float32)
    w = (1.0 / np.sqrt(deg_out[src] * deg_in[dst])).astype(np.float32)

    core = dst // NPC
    ldst = dst - core * NPC
    psrc = (src // NPC) * NPAD + (src % NPC)   # padded-global src row
    sb = psrc // P
    cg_e = sb // NBL
    lb_e = sb - cg_e * NBL
    si_e = np.asarray(SUB_OF)[lb_e]
    grp = cg_e * NSG + si_e
    j_e = lb_e - np.asarray(FIRSTB)[si_e]      # block index within group
    slot = psrc - sb * P

    order = np.lexsort((ldst, grp, core))
    c_s, g_s = core[order], grp[order]
    ld_s, j_s, sl_s, w_s = ldst[order], j_e[order], slot[order], w[order]
    key = c_s * NGRP + g_s
    starts = np.searchsorted(key, np.arange(NCORES * NGRP))
    ends = np.searchsorted(key, np.arange(NCORES * NGRP), side="right")

    # Tiles keep the group's sorted distinct rows; a tile whose rows are
    # all < EROW ("early") is gathered through a prefix-slice AP of the m
    # table, so it only depends on the early node blocks, which the
    # interleaved epilogue writes before the trailing collective ends.
    uniq_store = {}
    ntiles = np.ones(NGRP, np.int64)
    for p in range(NCORES):
        for g in range(NGRP):
            s0, s1 = starts[p * NGRP + g], ends[p * NGRP + g]
            if s1 > s0:
                uniq, inv = np.unique(ld_s[s0:s1], return_inverse=True)
            else:
                uniq, inv = np.zeros(1, np.int64), np.zeros(0, np.int64)
            uniq_store[p, g] = (uniq, inv, s0, s1)
            ntiles[g] = max(ntiles[g], -(-len(uniq) // P))
    subsz = np.asarray([SUBSZ[g % NSG] for g in range(NGRP)])
    colof = np.zeros(NGRP, np.int64)           # tile-column offsets (idx)
    colof[1:] = np.cumsum(ntiles)[:-1]
    czof = np.zeros(NGRP, np.int64)            # 128-col-block offsets (sel)
    czof[1:] = np.cumsum(ntiles * subsz)[:-1]
    nt = int(ntiles.sum())
    ncb = int((ntiles * subsz).sum())

    # early-tile count per group: tiles all of whose lanes (on EVERY core)
    # index rows < EROW
    ntA = ntiles.copy()
    for p in range(NCORES):
        for g in range(NGRP):
            uniq, inv, s0, s1 = uniq_store[p, g]
            na_rows = int((uniq < EROW).sum())
            ea = ntiles[g] if na_rows == len(uniq) else na_rows // P
            ntA[g] = min(ntA[g], ea)

    idx_tabs, sel_tabs = [], []
    for p in range(NCORES):
        idx_t = np.zeros((P, nt), np.int32)
        sel_t = np.zeros((P, ncb * P), np.float32)
        for g in range(NGRP):
            uniq, inv, s0, s1 = uniq_store[p, g]
            lanes_u = np.arange(len(uniq))
            idx_t[lanes_u % P, colof[g] + lanes_u // P] = uniq
            if s1 > s0:
                tile_e = inv // P
                lane_e = inv - tile_e * P
                cb = czof[g] + tile_e * subsz[g] + j_s[s0:s1]
                np.add.at(sel_t, (lane_e, cb * P + sl_s[s0:s1]), w_s[s0:s1])
        idx_tabs.append(idx_t)
        sel_tabs.append(sel_t.astype(BFNP))
    return [(int(n), int(a)) for n, a in zip(ntiles, ntA)], idx_tabs, sel_tabs


def _pack_lhsT(xT, kg):
    """[kg*128, NPAD] -> [128, kg*NPAD] (partition-major kg blocks)."""
    return np.ascontiguousarray(
        xT.reshape(kg, P, NPAD).transpose(1, 0, 2).reshape(P, kg * NPAD))


def _pack_rhs(Wm, kg, n):
    """[kg*128, n] -> [128, kg*n]."""
    return np.ascontiguousarray(
        Wm.reshape(kg, P, n).transpose(1, 0, 2).reshape(P, kg * n))


# ------------------------------------------------------------- device build
def _build(ntiles, repeat=1):
    ntiles_a = np.asarray([t[0] for t in ntiles])
    ntA = np.asarray([t[1] for t in ntiles])
    nt = int(ntiles_a.sum())
    ntmax = int(ntiles_a.max())
    subsz = np.asarray([SUBSZ[g % NSG] for g in range(NGRP)])
    colof = np.zeros(NGRP, np.int64)
    colof[1:] = np.cumsum(ntiles_a)[:-1]
    czof = np.zeros(NGRP, np.int64)
    czof[1:] = np.cumsum(ntiles_a * subsz)[:-1]
    ncb = int((ntiles_a * subsz).sum())
    nc = bass.Bass("TRN2", target_bir_lowering=False, debug=False,
                   num_devices=NCORES)

    def din(name, shape, dt):
        return nc.dram_tensor(name, shape, dt, kind="ExternalInput").ap()

    xT = din("xT", [P, KG1 * NPAD], BF)
    xrows = din("xrows", [NPAD, IN_F], BF)
    idx = din("idx", [P, nt], mybir.dt.int32)
    sel = din("sel", [P, ncb * P], BF)
    w1 = din("w1", [P, KG1 * CH], BF)
    v1 = din("v1", [P, KG1 * CH], BF)
    wk = din("wk", [P, 6 * KGC * CH], BF)
    vk = din("vk", [P, 6 * KGC * CH], BF)
    wd = din("wd", [P, KGC * N_LABELS], BF)
    ball = din("ball", [P, NLAYERS * CH], F32)
    bdr = din("bdr", [P, N_LABELS], F32)
    out = nc.dram_tensor("out", [NPAD, N_LABELS], F32, kind="ExternalOutput").ap()

    with tile.TileContext(nc) as tc:
        with (
            tc.tile_pool(name="const", bufs=1) as cp,
            tc.tile_pool(name="ht", bufs=2) as htp,
            tc.tile_pool(name="mout", bufs=2) as mp,
            tc.tile_pool(name="selg", bufs=3) as selp,
            tc.tile_pool(name="msg", bufs=30) as msgp,
            tc.tile_pool(name="pw", bufs=4) as pwp,
            tc.tile_pool(name="agg", bufs=4) as aggp,
            tc.tile_pool(name="hv", bufs=11) as hvp,
            tc.tile_pool(name="ep", bufs=3) as epp,
            tc.tile_pool(name="ut", bufs=2) as utp,
            tc.tile_pool(name="hact", bufs=2) as hp,
            tc.tile_pool(name="outs", bufs=2) as op,
            tc.tile_pool(name="psm", bufs=2, space="PSUM") as psm,
            tc.tile_pool(name="psagg", bufs=3, space="PSUM") as psagg,
            tc.tile_pool(name="pstr", bufs=2, space="PSUM") as pstr,
            tc.tile_pool(name="dram", bufs=1, space="DRAM") as dram,
        ):
            # ---- constants to SBUF, roughly in first-use order
            idx_t = cp.tile([P, nt], mybir.dt.int32)
            nc.sync.dma_start(idx_t[:], idx[:])
            w1_t = cp.tile([P, KG1 * CH], BF)
            nc.sync.dma_start(w1_t[:], w1[:])
            v1_t = cp.tile([P, KG1 * CH], BF)
            nc.sync.dma_start(v1_t[:], v1[:])
            xT_t = cp.tile([P, KG1 * NPAD], BF)
            nc.sync.dma_start(xT_t[:], xT[:])
            ball_t = cp.tile([P, NLAYERS * CH], F32)
            nc.sync.dma_start(ball_t[:], ball[:])
            wk_t = cp.tile([P, 6 * KGC * CH], BF)
            nc.sync.dma_start(wk_t[:], wk[:])
            vk_t = cp.tile([P, 6 * KGC * CH], BF)
            wd_t = cp.tile([P, KGC * N_LABELS], BF)
            bdr_t = cp.tile([P, N_LABELS], F32)
            ident = cp.tile([P, P], BF)
            make_identity(nc, ident[:])

            def scatter_phase(width, gsrc_full, gsrc_pref, rs_out, ptag,
                              mid_cb=None):
                """Grouped scatter.  Per chunk: first queue EVERY group's
                early-tile gathers (rows < EROW, via the prefix-slice AP,
                so they depend only on the early m-table blocks and can run
                under the previous layer's trailing collective), then per
                group stream the sel columns, gather the late tiles, and
                scatter-add into each block's accumulator.  After a chunk's
                partial writes, launch its ReduceScatter chunk.  mid_cb
                (the chunk-0 epilogue) is emitted near the end of chunk 1
                so its early m-table writes land before that collective
                finishes."""
                LOOKAHEAD = 6
                for ci, subids in enumerate(CHUNKS):
                    partial = dram.tile([NCORES, CHUNK_NB[ci] * P, width], BF,
                                        tag=f"{ptag}c{ci}")
                    seq = [cg * NSG + si
                           for cg in range(NCORES) for si in subids]
                    # fewest-late-tiles first: extends the early runway at
                    # the layer boundary before the first late-gather wait
                    seq.sort(key=lambda g: ntiles_a[g] - ntA[g])
                    gmsgs = {}

                    def emit_early(g):
                        co = int(colof[g])
                        msgs = []
                        for t in range(int(ntA[g])):
                            msg = msgp.tile([P, CH], BF, tag="msg")
                            nc.gpsimd.indirect_dma_start(
                                out=msg[:, :width], out_offset=None,
                                in_=gsrc_pref,
                                in_offset=bass.IndirectOffsetOnAxis(
                                    ap=idx_t[:, co + t:co + t + 1], axis=0))
                            msgs.append(msg)
                        gmsgs[g] = msgs

                    for k in range(min(LOOKAHEAD, len(seq))):
                        emit_early(seq[k])
                    for k, g in enumerate(seq):
                        if (ci == len(CHUNKS) - 1
                                and k == len(seq) - 1 - MID_BACK
                                and mid_cb is not None):
                            mid_cb()
                        if True:
                            cg, si = g // NSG, g % NSG
                            co, cz = int(colof[g]), int(czof[g])
                            ntg, S = int(ntiles_a[g]), SUBSZ[si]
                            selg = selp.tile([P, ntmax * SMAX * P], BF,
                                             tag="selg")
                            nc.scalar.dma_start(
                                selg[:, :ntg * S * P],
                                sel[:, cz * P:(cz + ntg * S) * P])
                            if k + LOOKAHEAD < len(seq):
                                emit_early(seq[k + LOOKAHEAD])
                            msgs = gmsgs[g]
                            for t in range(int(ntA[g]), ntg):
                                msg = msgp.tile([P, CH], BF, tag="msg")
                                nc.gpsimd.indirect_dma_start(
                                    out=msg[:, :width], out_offset=None,
                                    in_=gsrc_full,
                                    in_offset=bass.IndirectOffsetOnAxis(
                                        ap=idx_t[:, co + t:co + t + 1], axis=0))
                                msgs.append(msg)
                            row0 = (FIRSTB[si] - CHUNK_R0[ci]) * P
                            for j in range(S):
                                acc = psagg.tile([P, CH], F32, tag="agg")
                                for t in range(ntg):
                                    nc.tensor.matmul(
                                        acc[:, :width],
                                        selg[:, (t * S + j) * P:(t * S + j + 1) * P],
                                        msgs[t][:, :width],
                                        start=(t == 0), stop=(t == ntg - 1))
                                pwt = pwp.tile([P, CH], BF, tag="pw")
                                nc.vector.tensor_copy(pwt[:, :width],
                                                      acc[:, :width])
                                nc.sync.dma_start(
                                    partial[cg, row0 + j * P:
                                            row0 + (j + 1) * P, :],
                                    pwt[:, :width])
                    nc.gpsimd.collective_compute(
                        "ReduceScatter", mybir.AluOpType.add,
                        replica_groups=[list(range(NCORES))],
                        ins=[partial[:].opt()],
                        outs=[rs_out[CHUNK_R0[ci] * P:
                                     (CHUNK_R0[ci] + CHUNK_NB[ci]) * P,
                                     :].opt()])

            for rep in range(repeat):
                # ================= layer 1 (x-form) =================
                # hv1 = x @ V1 + b1, precomputed to SBUF (overlaps scatter/RS)
                hv_sb = []
                for b in range(NBL):
                    hps = psm.tile([P, CH], F32, tag="m")
                    for g in range(KG1):
                        nc.tensor.matmul(
                            hps[:],
                            xT_t[:, g * NPAD + b * P:g * NPAD + (b + 1) * P],
                            v1_t[:, g * CH:(g + 1) * CH],
                            start=(g == 0), stop=(g == KG1 - 1))
                    hvb = hvp.tile([P, CH], BF, tag="hv")
                    nc.vector.tensor_add(hvb[:], hps[:], ball_t[:, 0:CH])
                    hv_sb.append(hvb)

                rs1 = dram.tile([NPAD, IN_F], BF, tag=f"rs0_{rep}")

                def emit_m(hT, b, l_next, mtab_next):
                    """m = h @ W for node block b of the next conv layer,
                    written into its m-table half (A for blocks 0-6, B for
                    7-9, so the next layer's early gathers can start while
                    the trailing ReduceScatter chunk still runs)."""
                    wt = wk_t[:, (l_next - 1) * KGC * CH:l_next * KGC * CH]
                    mps = psm.tile([P, CH], F32, tag="m")
                    for g in range(KGC):
                        nc.tensor.matmul(
                            mps[:],
                            hT[:, g * NPAD + b * P:g * NPAD + (b + 1) * P],
                            wt[:, g * CH:(g + 1) * CH],
                            start=(g == 0), stop=(g == KGC - 1))
                    m_bf = mp.tile([P, CH], BF, tag="mbf")
                    nc.scalar.activation(
                        m_bf[:], mps[:], mybir.ActivationFunctionType.Copy)
                    nc.scalar.dma_start(
                        mtab_next[b * P:(b + 1) * P, :], m_bf[:])

                def emit_final(hT, b):
                    o_sb = op.tile([P, N_LABELS], F32, tag="o")
                    fps = []
                    for c in range(3):
                        fin_ps = psagg.tile([P, FIN_CHUNK], F32, tag="agg")
                        fps.append(fin_ps)
                    for g in range(KGC):
                        for c in range(3):
                            nc.tensor.matmul(
                                fps[c][:],
                                hT[:, g * NPAD + b * P:g * NPAD + (b + 1) * P],
                                wd_t[:, g * N_LABELS + c * FIN_CHUNK:
                                     g * N_LABELS + (c + 1) * FIN_CHUNK],
                                start=(g == 0), stop=(g == KGC - 1))
                    for c in range(3):
                        sl = slice(c * FIN_CHUNK, (c + 1) * FIN_CHUNK)
                        nc.vector.tensor_add(fps[c][:], fps[c][:], bdr_t[:, sl])
                        nc.scalar.activation(
                            o_sb[:, sl], fps[c][:],
                            mybir.ActivationFunctionType.Copy)
                    if rep == repeat - 1:
                        nc.sync.dma_start(out[b * P:(b + 1) * P, :], o_sb[:])

                def alloc_mtab(l):
                    mtab_n = dram.tile([NPAD, CH], BF, tag=f"mt{l % 2}")
                    return mtab_n

                hT_cur = htp.tile([P, KGC * NPAD], BF, tag="hT")
                mtab_next = alloc_mtab(1)

                def epi1_block(b):
                    u_sb = aggp.tile([P, CH], BF, tag="aggs")
                    nc.scalar.dma_start(u_sb[:, :IN_F],
                                        rs1[b * P:(b + 1) * P, :])
                    utt = utp.tile([P, IN_F], BF, tag="ut")
                    for g in range(KG1):
                        tr = pstr.tile([P, P], BF, tag="tr")
                        nc.tensor.transpose(
                            tr[:], u_sb[:, g * P:(g + 1) * P], ident[:])
                        nc.vector.tensor_copy(utt[:, g * P:(g + 1) * P], tr[:])
                    ups = psm.tile([P, CH], F32, tag="m")
                    for g in range(KG1):
                        nc.tensor.matmul(
                            ups[:], utt[:, g * P:(g + 1) * P],
                            w1_t[:, g * CH:(g + 1) * CH],
                            start=(g == 0), stop=(g == KG1 - 1))
                    ep = epp.tile([P, CH], BF, tag="ep")
                    nc.vector.tensor_add(ep[:], ups[:], hv_sb[b][:])
                    h_bf = hp.tile([P, CH], BF, tag="h")
                    nc.scalar.activation(
                        h_bf[:], ep[:], mybir.ActivationFunctionType.Relu)
                    for cg in range(KGC):
                        tr2 = pstr.tile([P, P], BF, tag="tr")
                        nc.tensor.transpose(
                            tr2[:], h_bf[:, cg * P:(cg + 1) * P], ident[:])
                        nc.vector.tensor_copy(
                            hT_cur[:, cg * NPAD + b * P:cg * NPAD + (b + 1) * P],
                            tr2[:])
                    emit_m(hT_cur, b, 1, mtab_next)

                scatter_phase(IN_F, xrows[:], xrows[:EROW], rs1, f"ptx{rep}",
                              mid_cb=lambda: [epi1_block(b)
                                              for b in range(CHUNK_NB[0])])
                if rep == 0:
                    nc.sync.dma_start(vk_t[:], vk[:])
                    nc.sync.dma_start(wd_t[:], wd[:])
                    nc.sync.dma_start(bdr_t[:], bdr[:])
                for b in range(CHUNK_NB[0], NBL):
                    epi1_block(b)

                # ================= layers 2..7 (m-form) =================
                for l in range(1, NLAYERS):
                    vt = vk_t[:, (l - 1) * KGC * CH:l * KGC * CH]
                    mtab = mtab_next
                    rs = dram.tile([NPAD, CH], BF, tag=f"rs{l}_{rep}")
                    if l < NLAYERS - 1:
                        mtab_next = alloc_mtab(l + 1)
                    hT_next = htp.tile([P, KGC * NPAD], BF, tag="hT")

                    # epilogue: per block hv, add+relu, transposes, then the
                    # next layer's m (or the final dense) for that block;
                    # chunk-0 blocks interleave with the scatter tail so
                    # m-table A lands before the trailing collective
                    def epi_block(b, l=l, vt=vt, rs=rs, hT_cur=hT_cur,
                                  hT_next=hT_next, mtab_next=mtab_next):
                        hps = psm.tile([P, CH], F32, tag="m")
                        for g in range(KGC):
                            nc.tensor.matmul(
                                hps[:],
                                hT_cur[:, g * NPAD + b * P:g * NPAD + (b + 1) * P],
                                vt[:, g * CH:(g + 1) * CH],
                                start=(g == 0), stop=(g == KGC - 1))
                        hvb = hvp.tile([P, CH], BF, tag="hv")
                        nc.vector.tensor_add(
                            hvb[:], hps[:], ball_t[:, l * CH:(l + 1) * CH])
                        agg_sb = aggp.tile([P, CH], BF, tag="aggs")
                        nc.scalar.dma_start(agg_sb[:], rs[b * P:(b + 1) * P, :])
                        ep = epp.tile([P, CH], BF, tag="ep")
                        nc.vector.tensor_add(ep[:], agg_sb[:], hvb[:])
                        h_bf = hp.tile([P, CH], BF, tag="h")
                        nc.scalar.activation(
                            h_bf[:], ep[:], mybir.ActivationFunctionType.Relu)
                        for cg in range(KGC):
                            tr2 = pstr.tile([P, P], BF, tag="tr")
                            nc.tensor.transpose(
                                tr2[:], h_bf[:, cg * P:(cg + 1) * P], ident[:])
                            nc.vector.tensor_copy(
                                hT_next[:, cg * NPAD + b * P:cg * NPAD + (b + 1) * P],
                                tr2[:])
                        if l < NLAYERS - 1:
                            emit_m(hT_next, b, l + 1, mtab_next)
                        else:
                            emit_final(hT_next, b)

                    scatter_phase(CH, mtab[:], mtab[:EROW], rs,
                                  f"pt{l % 2}_{rep}",
                                  mid_cb=lambda epi_block=epi_block:
                                      [epi_block(b) for b in range(CHUNK_NB[0])])
                    for b in range(CHUNK_NB[0], NBL):
                        epi_block(b)
                    hT_cur = hT_next

    _split_excess_waits(nc)
    return nc


# ------------------------------------------------------------- entry point
def kernel(x, src, dst, W1, V1, b1, Wk, Vk, bk, Wd, bd, _repeat=1, _nc_cache={}):
    x = np.asarray(x, np.float32)
    ntiles, idx_tabs, sel_tabs = _prep_edges(src, dst)

    key = (tuple(ntiles), _repeat)
    if key not in _nc_cache:
        _nc_cache[key] = _build(ntiles, repeat=_repeat)
    nc = _nc_cache[key]

    # weights (replicated, host-packed)
    w1p = _pack_rhs(np.asarray(W1, np.float32), KG1, CH).astype(BFNP)
    v1p = _pack_rhs(np.asarray(V1, np.float32), KG1, CH).astype(BFNP)
    wkp = np.concatenate(
        [_pack_rhs(np.asarray(Wk[i], np.float32), KGC, CH) for i in range(6)],
        axis=1).astype(BFNP)
    vkp = np.concatenate(
        [_pack_rhs(np.asarray(Vk[i], np.float32), KGC, CH) for i in range(6)],
        axis=1).astype(BFNP)
    wdp = _pack_rhs(np.asarray(Wd, np.float32), KGC, N_LABELS).astype(BFNP)
    ballv = np.concatenate(
        [np.asarray(b1, np.float32)] + [np.asarray(bk[i], np.float32)
                                        for i in range(6)])
    ballp = np.broadcast_to(ballv, (P, NLAYERS * CH)).copy()
    bdp = np.broadcast_to(np.asarray(bd, np.float32), (P, N_LABELS)).copy()

    in_maps = []
    for p in range(NCORES):
        xp = np.zeros((NPAD, IN_F), np.float32)
        xp[:NPC] = x[p * NPC:(p + 1) * NPC]
        xTp = _pack_lhsT(np.ascontiguousarray(xp.T), KG1).astype(BFNP)
        xbf = xp.astype(BFNP)
        in_maps.append({
            "xT": xTp, "xrows": xbf,
            "idx": idx_tabs[p], "sel": sel_tabs[p],
            "w1": w1p, "v1": v1p, "wk": wkp, "vk": vkp, "wd": wdp,
            "ball": ballp, "bdr": bdp,
        })

    res = run_bass_kernel_spmd(nc, in_maps, core_ids=list(range(NCORES)))
    outp = np.empty((N_NODES, N_LABELS), np.float32)
    for p in range(NCORES):
        outp[p * NPC:(p + 1) * NPC] = res.results[p]["out"][:NPC]
    return outp


# revision 61
# speedup vs baseline: 1.3072x; 1.0715x over previous
"""ARMA-style GNN message passing on 8 TRN2 NeuronCores.

Reference computation (per layer, 7 layers):
    m   = h @ W                                  [N, CH]
    agg = segment_sum(w[:,None] * m[dst], src)   [N, CH]
    h'  = relu(agg + h @ V + b)
then logits = h @ Wd + bd.

Strategy (graph/data parallel over nodes, ReduceScatter formulation):
  - 8 cores own 1250 nodes each (padded to 1280 = 10 blocks of 128).
  - Edge (s, d) is processed by the core owning d (where the message row
    m[d] is LOCAL).  Each core computes, per layer, a partial aggregation
    table over the full padded-global src space [8*1280, CH]:
        partial_p[s] = sum_{e: dst_e in p, src_e = s} w_e * m_local[dst_e]
    then one ReduceScatter(add) sums the 8 partials and hands every core
    the [1280, CH] slice for its own nodes.  (A ReduceScatter's output is
    1/8 the size of the AllGather the src-side formulation needs, which is
    what the inter-chip collective cost scales with.)
  - Edges are bucketed by global src block (80 buckets of 128 src slots).
    Within a bucket the distinct local dst rows are deduplicated into
    gather tiles of 128 rows; one indirect DMA fetches a tile from the
    local m table in DRAM, and a [128 dst-rows x 128 src-slots] bf16
    "selection" matrix carrying the degree weights w_e scatter-adds it
    into the bucket's PSUM accumulator on PE.
  - Layer 1 scatters x itself (256 wide, half the traffic) and applies W1
    after the ReduceScatter: A@(x W1) == (A@x) W1.
  - h@V + b is precomputed into SBUF while the ReduceScatter is in
    flight; the post-collective epilogue is add + relu + PE-transpose.
  - Final dense layer and output assembly per core; host concatenates.

All matmuls run in bf16 with fp32 PSUM accumulation.
"""
import numpy as np
import ml_dtypes

import concourse.bass as bass
import concourse.tile as tile
import concourse.mybir as mybir
from concourse.vector_clock import ScopedClock
from concourse.bass_utils import run_bass_kernel_spmd
from concourse.masks import make_identity

# ---------------------------------------------------------------- constants
N_NODES = 10000
N_EDGES = 160000
IN_F = 256
CH = 512
N_LABELS = 1440
NCORES = 8
NPC = N_NODES // NCORES      # 1250 nodes per core
P = 128
NBL = 10                     # node blocks per core (10*128 = 1280)
NPAD = NBL * P               # padded nodes per core
NSB = NCORES * NBL           # global src blocks (80)
NGPAD = NCORES * NPAD        # padded global node space (10240)
NLAYERS = 7
KG1 = IN_F // P              # 2 contraction blocks in layer 1
KGC = CH // P                # 4 contraction blocks in layers 2..7
FIN_CHUNK = 480              # 1440 = 3 * 480, fits one PSUM bank in f32

BF = mybir.dt.bfloat16
F32 = mybir.dt.float32
BFNP = ml_dtypes.bfloat16


# ------------------------------------------------------- walrus workarounds
def _patched_drain_and_barrier(self, tick_clock, wait_clock):
    # This walrus build rejects >1-2 sync waits on one TPB_CTRL; put the
    # kernel-tail drain's waits on separate preceding SP nops instead.
    nc = self.nc
    probe = nc.sync.nop(nofuse=True, hint="drain_waits")
    wait_clock.add_sem_waits(probe.ins, ScopedClock({None: tick_clock.global_clock}))
    si = probe.ins.sync_info
    waits = list(si.on_wait) if si is not None else []
    if len(waits) > 1:
        si.on_wait = waits[:1]
        for i in range(1, len(waits)):
            n2 = nc.sync.nop(nofuse=True, hint=f"drain_waits_{i}")
            n2.ins.sync_info = mybir.SyncInfo(on_wait=[waits[i]], on_update=[])
    nc.sync.drain()
    nc.all_engine_barrier()
    assert self.sems is not None
    popped = nc._tile_sem_poison_stack.pop()
    assert popped is self._sem_poison
    nc.clear_and_free_semaphores(list(self.sems.allocated().values()))
    nc.all_engine_barrier()


tile.TileContext._drain_and_barrier = _patched_drain_and_barrier


def _split_excess_waits(nc, limit=1):
    # Same ISA restriction for ordinary instructions: hoist excess sync
    # waits onto injected same-engine nops placed just before.
    for func in nc.m.functions:
        for bb in func.blocks:
            out = []
            for ins in bb.instructions:
                si = ins.sync_info
                if si is not None and si.on_wait and len(si.on_wait) > limit:
                    waits = list(si.on_wait)
                    excess, keep = waits[:-limit], waits[-limit:]
                    for i in range(0, len(excess), limit):
                        out.append(mybir.InstNoOp(
                            name=f"{ins.name}_xw{i}",
                            engine=ins.engine,
                            ins=[], outs=[],
                            sync_info=mybir.SyncInfo(
                                on_wait=excess[i:i + limit], on_update=[]),
                        ))
                    si.on_wait = keep
                out.append(ins)
            bb.instructions[:] = out


# ------------------------------------------------------------- host prep
SUBGROUPS = ((0, 1, 2, 3), (4, 5, 6), (7, 8, 9))   # local src blocks per group
NSG = len(SUBGROUPS)
NGRP = NCORES * NSG           # 24 groups; group g = (core g//NSG, sub g%NSG)
CHUNKS = ((0, 1), (2,))       # subgroup ids per ReduceScatter chunk
CHUNK_R0 = (0, 7)             # first local block of each chunk
CHUNK_NB = (7, 3)             # local blocks per chunk
SUBSZ = tuple(len(s) for s in SUBGROUPS)
SUB_OF = [si for si, s in enumerate(SUBGROUPS) for _ in s]
FIRSTB = tuple(s[0] for s in SUBGROUPS)
SMAX = max(SUBSZ)
EROW = CHUNK_NB[0] * P        # gather rows below this are written early
MID_BACK = 6                  # hv insert point: groups before chunk-1 end


def _prep_edges(src, dst):
    """Route each edge to the core owning dst.  The 80 global src blocks
    form 24 groups (per src-owning core: blocks 0-3, 4-6, 7-9; the 7/3
    block split matches the two ReduceScatter chunks).  Per group the
    distinct local dst rows are deduplicated into gather tiles of 128 and
    gathered once; each tile then feeds one sel-matmul per block in the
    group.  Returns (ntiles, idx_tabs, sel_tabs): ntiles[g] = tile count
    of group g (max over cores, same program on all cores); per core an
    idx table [128, NT] of local m-table rows and a sel table of bf16
    degree weights, column blocks ordered (group, tile, block-in-group)."""
    src = np.asarray(src).astype(np.int64)
    dst = np.asarray(dst).astype(np.int64)
    deg_out = np.maximum(np.bincount(src, minlength=N_NODES), 1.0).astype(np.float32)
    deg_in = np.maximum(np.bincount(dst, minlength=N_NODES), 1.0).astype(np.float32)
    w = (1.0 / np.sqrt(deg_out[src] * deg_in[dst])).astype(np.float32)

    core = dst // NPC
    ldst = dst - core * NPC
    psrc = (src // NPC) * NPAD + (src % NPC)   # padded-global src row
    sb = psrc // P
    cg_e = sb // NBL
    lb_e = sb - cg_e * NBL
    si_e = np.asarray(SUB_OF)[lb_e]
    grp = cg_e * NSG + si_e
    j_e = lb_e - np.asarray(FIRSTB)[si_e]      # block index within group
    slot = psrc - sb * P

    order = np.lexsort((ldst, grp, core))
    c_s, g_s = core[order], grp[order]
    ld_s, j_s, sl_s, w_s = ldst[order], j_e[order], slot[order], w[order]
    key = c_s * NGRP + g_s
    starts = np.searchsorted(key, np.arange(NCORES * NGRP))
    ends = np.searchsorted(key, np.arange(NCORES * NGRP), side="right")

    # Tiles keep the group's sorted distinct rows; a tile whose rows are
    # all < EROW ("early") is gathered through a prefix-slice AP of the m
    # table, so it only depends on the early node blocks, which the
    # interleaved epilogue writes before the trailing collective ends.
    uniq_store = {}
    ntiles = np.ones(NGRP, np.int64)
    for p in range(NCORES):
        for g in range(NGRP):
            s0, s1 = starts[p * NGRP + g], ends[p * NGRP + g]
            if s1 > s0:
                uniq, inv = np.unique(ld_s[s0:s1], return_inverse=True)
            else:
                uniq, inv = np.zeros(1, np.int64), np.zeros(0, np.int64)
            uniq_store[p, g] = (uniq, inv, s0, s1)
            ntiles[g] = max(ntiles[g], -(-len(uniq) // P))
    subsz = np.asarray([SUBSZ[g % NSG] for g in range(NGRP)])
    colof = np.zeros(NGRP, np.int64)           # tile-column offsets (idx)
    colof[1:] = np.cumsum(ntiles)[:-1]
    czof = np.zeros(NGRP, np.int64)            # 128-col-block offsets (sel)
    czof[1:] = np.cumsum(ntiles * subsz)[:-1]
    nt = int(ntiles.sum())
    ncb = int((ntiles * subsz).sum())

    # early-tile count per group: tiles all of whose lanes (on EVERY core)
    # index rows < EROW
    ntA = ntiles.copy()
    for p in range(NCORES):
        for g in range(NGRP):
            uniq, inv, s0, s1 = uniq_store[p, g]
            na_rows = int((uniq < EROW).sum())
            ea = ntiles[g] if na_rows == len(uniq) else na_rows // P
            ntA[g] = min(ntA[g], ea)

    idx_tabs, sel_tabs = [], []
    for p in range(NCORES):
        idx_t = np.zeros((P, nt), np.int32)
        sel_t = np.zeros((P, ncb * P), np.float32)
        for g in range(NGRP):
            uniq, inv, s0, s1 = uniq_store[p, g]
            lanes_u = np.arange(len(uniq))
            idx_t[lanes_u % P, colof[g] + lanes_u // P] = uniq
            if s1 > s0:
                tile_e = inv // P
                lane_e = inv - tile_e * P
                cb = czof[g] + tile_e * subsz[g] + j_s[s0:s1]
                np.add.at(sel_t, (lane_e, cb * P + sl_s[s0:s1]), w_s[s0:s1])
        idx_tabs.append(idx_t)
        sel_tabs.append(sel_t.astype(BFNP))
    return [(int(n), int(a)) for n, a in zip(ntiles, ntA)], idx_tabs, sel_tabs


def _pack_lhsT(xT, kg):
    """[kg*128, NPAD] -> [128, kg*NPAD] (partition-major kg blocks)."""
    return np.ascontiguousarray(
        xT.reshape(kg, P, NPAD).transpose(1, 0, 2).reshape(P, kg * NPAD))


def _pack_rhs(Wm, kg, n):
    """[kg*128, n] -> [128, kg*n]."""
    return np.ascontiguousarray(
        Wm.reshape(kg, P, n).transpose(1, 0, 2).reshape(P, kg * n))


# ------------------------------------------------------------- device build
def _build(ntiles, repeat=1):
    ntiles_a = np.asarray([t[0] for t in ntiles])
    ntA = np.asarray([t[1] for t in ntiles])
    nt = int(ntiles_a.sum())
    ntmax = int(ntiles_a.max())
    subsz = np.asarray([SUBSZ[g % NSG] for g in range(NGRP)])
    colof = np.zeros(NGRP, np.int64)
    colof[1:] = np.cumsum(ntiles_a)[:-1]
    czof = np.zeros(NGRP, np.int64)
    czof[1:] = np.cumsum(ntiles_a * subsz)[:-1]
    ncb = int((ntiles_a * subsz).sum())
    nc = bass.Bass("TRN2", target_bir_lowering=False, debug=False,
                   num_devices=NCORES)

    def din(name, shape, dt):
        return nc.dram_tensor(name, shape, dt, kind="ExternalInput").ap()

    xT = din("xT", [P, KG1 * NPAD], BF)
    xrows = din("xrows", [NPAD, IN_F], BF)
    idx = din("idx", [P, nt], mybir.dt.int32)
    sel = din("sel", [P, ncb * P], BF)
    w1 = din("w1", [P, KG1 * CH], BF)
    v1 = din("v1", [P, KG1 * CH], BF)
    wk = din("wk", [P, 6 * KGC * CH], BF)
    vk = din("vk", [P, 6 * KGC * CH], BF)
    wd = din("wd", [P, KGC * N_LABELS], BF)
    ball = din("ball", [P, NLAYERS * CH], F32)
    bdr = din("bdr", [P, N_LABELS], F32)
    out = nc.dram_tensor("out", [NPAD, N_LABELS], F32, kind="ExternalOutput").ap()

    with tile.TileContext(nc) as tc:
        with (
            tc.tile_pool(name="const", bufs=1) as cp,
            tc.tile_pool(name="ht", bufs=2) as htp,
            tc.tile_pool(name="mout", bufs=2) as mp,
            tc.tile_pool(name="selg", bufs=3) as selp,
            tc.tile_pool(name="msg", bufs=30) as msgp,
            tc.tile_pool(name="pw", bufs=4) as pwp,
            tc.tile_pool(name="agg", bufs=4) as aggp,
            tc.tile_pool(name="hv", bufs=11) as hvp,
            tc.tile_pool(name="ep", bufs=3) as epp,
            tc.tile_pool(name="ut", bufs=2) as utp,
            tc.tile_pool(name="hact", bufs=2) as hp,
            tc.tile_pool(name="outs", bufs=2) as op,
            tc.tile_pool(name="psm", bufs=2, space="PSUM") as psm,
            tc.tile_pool(name="psagg", bufs=3, space="PSUM") as psagg,
            tc.tile_pool(name="pstr", bufs=2, space="PSUM") as pstr,
            tc.tile_pool(name="dram", bufs=1, space="DRAM") as dram,
        ):
            # ---- constants to SBUF, roughly in first-use order
            idx_t = cp.tile([P, nt], mybir.dt.int32)
            nc.sync.dma_start(idx_t[:], idx[:])
            w1_t = cp.tile([P, KG1 * CH], BF)
            nc.sync.dma_start(w1_t[:], w1[:])
            v1_t = cp.tile([P, KG1 * CH], BF)
            nc.sync.dma_start(v1_t[:], v1[:])
            xT_t = cp.tile([P, KG1 * NPAD], BF)
            nc.sync.dma_start(xT_t[:], xT[:])
            ball_t = cp.tile([P, NLAYERS * CH], F32)
            nc.sync.dma_start(ball_t[:], ball[:])
            wk_t = cp.tile([P, 6 * KGC * CH], BF)
            nc.sync.dma_start(wk_t[:], wk[:])
            vk_t = cp.tile([P, 6 * KGC * CH], BF)
            wd_t = cp.tile([P, KGC * N_LABELS], BF)
            bdr_t = cp.tile([P, N_LABELS], F32)
            ident = cp.tile([P, P], BF)
            make_identity(nc, ident[:])

            def scatter_phase(width, gsrc_full, gsrc_pref, rs_out, ptag,
                              mid_cb=None):
                """Grouped scatter.  Per chunk: first queue EVERY group's
                early-tile gathers (rows < EROW, via the prefix-slice AP,
                so they depend only on the early m-table blocks and can run
                under the previous layer's trailing collective), then per
                group stream the sel columns, gather the late tiles, and
                scatter-add into each block's accumulator.  After a chunk's
                partial writes, launch its ReduceScatter chunk.  mid_cb
                (the chunk-0 epilogue) is emitted near the end of chunk 1
                so its early m-table writes land before that collective
                finishes."""
                LOOKAHEAD = 6
                for ci, subids in enumerate(CHUNKS):
                    partial = dram.tile([NCORES, CHUNK_NB[ci] * P, width], BF,
                                        tag=f"{ptag}c{ci}")
                    seq = [cg * NSG + si
                           for cg in range(NCORES) for si in subids]
                    # fewest-late-tiles first: extends the early runway at
                    # the layer boundary before the first late-gather wait
                    seq.sort(key=lambda g: ntiles_a[g] - ntA[g])
                    gmsgs = {}

                    def emit_early(g):
                        co = int(colof[g])
                        msgs = []
                        for t in range(int(ntA[g])):
                            msg = msgp.tile([P, CH], BF, tag="msg")
                            nc.gpsimd.indirect_dma_start(
                                out=msg[:, :width], out_offset=None,
                                in_=gsrc_pref,
                                in_offset=bass.IndirectOffsetOnAxis(
                                    ap=idx_t[:, co + t:co + t + 1], axis=0))
                            msgs.append(msg)
                        gmsgs[g] = msgs

                    for k in range(min(LOOKAHEAD, len(seq))):
                        emit_early(seq[k])
                    for k, g in enumerate(seq):
                        if (ci == len(CHUNKS) - 1
                                and k == len(seq) - 1 - MID_BACK
                                and mid_cb is not None):
                            mid_cb()
                        if True:
                            cg, si = g // NSG, g % NSG
                            co, cz = int(colof[g]), int(czof[g])
                            ntg, S = int(ntiles_a[g]), SUBSZ[si]
                            selg = selp.tile([P, ntmax * SMAX * P], BF,
                                             tag="selg")
                            nc.scalar.dma_start(
                                selg[:, :ntg * S * P],
                                sel[:, cz * P:(cz + ntg * S) * P])
                            if k + LOOKAHEAD < len(seq):
                                emit_early(seq[k + LOOKAHEAD])
                            msgs = gmsgs[g]
                            for t in range(int(ntA[g]), ntg):
                                msg = msgp.tile([P, CH], BF, tag="msg")
                                nc.gpsimd.indirect_dma_start(
                                    out=msg[:, :width], out_offset=None,
                                    in_=gsrc_full,
                                    in_offset=bass.IndirectOffsetOnAxis(
                                        ap=idx_t[:, co + t:co + t + 1], axis=0))
                                msgs.append(msg)
                            row0 = (FIRSTB[si] - CHUNK_R0[ci]) * P
                            for j in range(S):
                                acc = psagg.tile([P, CH], F32, tag="agg")
                                for t in range(ntg):
                                    nc.tensor.matmul(
                                        acc[:, :width],
                                        selg[:, (t * S + j) * P:(t * S + j + 1) * P],
                                        msgs[t][:, :width],
                                        start=(t == 0), stop=(t == ntg - 1))
                                pwt = pwp.tile([P, CH], BF, tag="pw")
                                nc.vector.tensor_copy(pwt[:, :width],
                                                      acc[:, :width])
                                nc.sync.dma_start(
                                    partial[cg, row0 + j * P:
                                            row0 + (j + 1) * P, :],
                                    pwt[:, :width])
                    nc.gpsimd.collective_compute(
                        "ReduceScatter", mybir.AluOpType.add,
                        replica_groups=[list(range(NCORES))],
                        ins=[partial[:].opt()],
                        outs=[rs_out[CHUNK_R0[ci] * P:
                                     (CHUNK_R0[ci] + CHUNK_NB[ci]) * P,
                                     :].opt()])

            for rep in range(repeat):
                # ================= layer 1 (x-form) =================
                # hv1 = x @ V1 + b1, precomputed to SBUF (overlaps scatter/RS)
                hv_sb = []
                for b in range(NBL):
                    hps = psm.tile([P, CH], F32, tag="m")
                    for g in range(KG1):
                        nc.tensor.matmul(
                            hps[:],
                            xT_t[:, g * NPAD + b * P:g * NPAD + (b + 1) * P],
                            v1_t[:, g * CH:(g + 1) * CH],
                            start=(g == 0), stop=(g == KG1 - 1))
                    hvb = hvp.tile([P, CH], BF, tag="hv")
                    nc.vector.tensor_add(hvb[:], hps[:], ball_t[:, 0:CH])
                    hv_sb.append(hvb)

                rs1 = dram.tile([NPAD, IN_F], BF, tag=f"rs0_{rep}")

                def emit_m(hT, b, l_next, mtab_next):
                    """m = h @ W for node block b of the next conv layer,
                    written into its m-table half (A for blocks 0-6, B for
                    7-9, so the next layer's early gathers can start while
                    the trailing ReduceScatter chunk still runs)."""
                    wt = wk_t[:, (l_next - 1) * KGC * CH:l_next * KGC * CH]
                    mps = psm.tile([P, CH], F32, tag="m")
                    for g in range(KGC):
                        nc.tensor.matmul(
                            mps[:],
                            hT[:, g * NPAD + b * P:g * NPAD + (b + 1) * P],
                            wt[:, g * CH:(g + 1) * CH],
                            start=(g == 0), stop=(g == KGC - 1))
                    m_bf = mp.tile([P, CH], BF, tag="mbf")
                    nc.scalar.activation(
                        m_bf[:], mps[:], mybir.ActivationFunctionType.Copy)
                    nc.scalar.dma_start(
                        mtab_next[b * P:(b + 1) * P, :], m_bf[:])

                def emit_final(hT, b):
                    o_sb = op.tile([P, N_LABELS], F32, tag="o")
                    fps = []
                    for c in range(3):
                        fin_ps = psagg.tile([P, FIN_CHUNK], F32, tag="agg")
                        fps.append(fin_ps)
                    for g in range(KGC):
                        for c in range(3):
                            nc.tensor.matmul(
                                fps[c][:],
                                hT[:, g * NPAD + b * P:g * NPAD + (b + 1) * P],
                                wd_t[:, g * N_LABELS + c * FIN_CHUNK:
                                     g * N_LABELS + (c + 1) * FIN_CHUNK],
                                start=(g == 0), stop=(g == KGC - 1))
                    for c in range(3):
                        sl = slice(c * FIN_CHUNK, (c + 1) * FIN_CHUNK)
                        nc.vector.tensor_add(fps[c][:], fps[c][:], bdr_t[:, sl])
                        nc.scalar.activation(
                            o_sb[:, sl], fps[c][:],
                            mybir.ActivationFunctionType.Copy)
                    if rep == repeat - 1:
                        nc.sync.dma_start(out[b * P:(b + 1) * P, :], o_sb[:])

                def alloc_mtab(l):
                    mtab_n = dram.tile([NPAD, CH], BF, tag=f"mt{l % 2}")
                    return mtab_n

                hT_cur = htp.tile([P, KGC * NPAD], BF, tag="hT")
                mtab_next = alloc_mtab(1)

                def epi1_block(b):
                    u_sb = aggp.tile([P, CH], BF, tag="aggs")
                    nc.scalar.dma_start(u_sb[:, :IN_F],
                                        rs1[b * P:(b + 1) * P, :])
                    utt = utp.tile([P, IN_F], BF, tag="ut")
                    for g in range(KG1):
                        tr = pstr.tile([P, P], BF, tag="tr")
                        nc.tensor.transpose(
                            tr[:], u_sb[:, g * P:(g + 1) * P], ident[:])
                        nc.vector.tensor_copy(utt[:, g * P:(g + 1) * P], tr[:])
                    ups = psm.tile([P, CH], F32, tag="m")
                    for g in range(KG1):
                        nc.tensor.matmul(
                            ups[:], utt[:, g * P:(g + 1) * P],
                            w1_t[:, g * CH:(g + 1) * CH],
                            start=(g == 0), stop=(g == KG1 - 1))
                    ep = epp.tile([P, CH], BF, tag="ep")
                    nc.vector.tensor_add(ep[:], ups[:], hv_sb[b][:])
                    h_bf = hp.tile([P, CH], BF, tag="h")
                    nc.scalar.activation(
                        h_bf[:], ep[:], mybir.ActivationFunctionType.Relu)
                    for cg in range(KGC):
                        tr2 = pstr.tile([P, P], BF, tag="tr")
                        nc.tensor.transpose(
                            tr2[:], h_bf[:, cg * P:(cg + 1) * P], ident[:])
                        nc.vector.tensor_copy(
                            hT_cur[:, cg * NPAD + b * P:cg * NPAD + (b + 1) * P],
                            tr2[:])
                    emit_m(hT_cur, b, 1, mtab_next)

                scatter_phase(IN_F, xrows[:], xrows[:EROW], rs1, f"ptx{rep}")
                if rep == 0:
                    nc.sync.dma_start(vk_t[:], vk[:])
                    nc.sync.dma_start(wd_t[:], wd[:])
                    nc.sync.dma_start(bdr_t[:], bdr[:])
                for b in range(NBL):
                    epi1_block(b)

                # ================= layers 2..7 (m-form) =================
                for l in range(1, NLAYERS):
                    vt = vk_t[:, (l - 1) * KGC * CH:l * KGC * CH]
                    mtab = mtab_next
                    rs = dram.tile([NPAD, CH], BF, tag=f"rs{l}_{rep}")
                    if l < NLAYERS - 1:
                        mtab_next = alloc_mtab(l + 1)
                    hT_next = htp.tile([P, KGC * NPAD], BF, tag="hT")

                    # epilogue: per block hv, add+relu, transposes, then the
                    # next layer's m (or the final dense) for that block;
                    # chunk-0 blocks interleave with the scatter tail so
                    # m-table A lands before the trailing collective
                    hvbs = {}

                    def hv_block(b, l=l, vt=vt, hT_cur=hT_cur):
                        hps = psm.tile([P, CH], F32, tag="m")
                        for g in range(KGC):
                            nc.tensor.matmul(
                                hps[:],
                                hT_cur[:, g * NPAD + b * P:g * NPAD + (b + 1) * P],
                                vt[:, g * CH:(g + 1) * CH],
                                start=(g == 0), stop=(g == KGC - 1))
                        hvb = hvp.tile([P, CH], BF, tag="hv")
                        nc.vector.tensor_add(
                            hvb[:], hps[:], ball_t[:, l * CH:(l + 1) * CH])
                        hvbs[b] = hvb

                    def epi_rest(b, l=l, rs=rs, hT_next=hT_next,
                                 mtab_next=mtab_next):
                        agg_sb = aggp.tile([P, CH], BF, tag="aggs")
                        nc.scalar.dma_start(agg_sb[:], rs[b * P:(b + 1) * P, :])
                        ep = epp.tile([P, CH], BF, tag="ep")
                        nc.vector.tensor_add(ep[:], agg_sb[:], hvbs[b][:])
                        h_bf = hp.tile([P, CH], BF, tag="h")
                        nc.scalar.activation(
                            h_bf[:], ep[:], mybir.ActivationFunctionType.Relu)
                        for cg in range(KGC):
                            tr2 = pstr.tile([P, P], BF, tag="tr")
                            nc.tensor.transpose(
                                tr2[:], h_bf[:, cg * P:(cg + 1) * P], ident[:])
                            nc.vector.tensor_copy(
                                hT_next[:, cg * NPAD + b * P:cg * NPAD + (b + 1) * P],
                                tr2[:])
                        if l < NLAYERS - 1:
                            emit_m(hT_next, b, l + 1, mtab_next)
                        else:
                            emit_final(hT_next, b)

                    scatter_phase(CH, mtab[:], mtab[:EROW], rs,
                                  f"pt{l % 2}_{rep}",
                                  mid_cb=lambda hv_block=hv_block:
                                      [hv_block(b) for b in range(NBL)])
                    for b in range(NBL):
                        epi_rest(b)
                    hT_cur = hT_next

    _split_excess_waits(nc)
    return nc


# ------------------------------------------------------------- entry point
def kernel(x, src, dst, W1, V1, b1, Wk, Vk, bk, Wd, bd, _repeat=1, _nc_cache={}):
    x = np.asarray(x, np.float32)
    ntiles, idx_tabs, sel_tabs = _prep_edges(src, dst)

    key = (tuple(ntiles), _repeat)
    if key not in _nc_cache:
        _nc_cache[key] = _build(ntiles, repeat=_repeat)
    nc = _nc_cache[key]

    # weights (replicated, host-packed)
    w1p = _pack_rhs(np.asarray(W1, np.float32), KG1, CH).astype(BFNP)
    v1p = _pack_rhs(np.asarray(V1, np.float32), KG1, CH).astype(BFNP)
    wkp = np.concatenate(
        [_pack_rhs(np.asarray(Wk[i], np.float32), KGC, CH) for i in range(6)],
        axis=1).astype(BFNP)
    vkp = np.concatenate(
        [_pack_rhs(np.asarray(Vk[i], np.float32), KGC, CH) for i in range(6)],
        axis=1).astype(BFNP)
    wdp = _pack_rhs(np.asarray(Wd, np.float32), KGC, N_LABELS).astype(BFNP)
    ballv = np.concatenate(
        [np.asarray(b1, np.float32)] + [np.asarray(bk[i], np.float32)
                                        for i in range(6)])
    ballp = np.broadcast_to(ballv, (P, NLAYERS * CH)).copy()
    bdp = np.broadcast_to(np.asarray(bd, np.float32), (P, N_LABELS)).copy()

    in_maps = []
    for p in range(NCORES):
        xp = np.zeros((NPAD, IN_F), np.float32)
        xp[:NPC] = x[p * NPC:(p + 1) * NPC]
        xTp = _pack_lhsT(np.ascontiguousarray(xp.T), KG1).astype(BFNP)
        xbf = xp.astype(BFNP)
        in_maps.append({
            "xT": xTp, "xrows": xbf,
            "idx": idx_tabs[p], "sel": sel_tabs[p],
            "w1": w1p, "v1": v1p, "wk": wkp, "vk": vkp, "wd": wdp,
            "ball": ballp, "bdr": bdp,
        })

    res = run_bass_kernel_spmd(nc, in_maps, core_ids=list(range(NCORES)))
    outp = np.empty((N_NODES, N_LABELS), np.float32)
    for p in range(NCORES):
        outp[p * NPC:(p + 1) * NPC] = res.results[p]["out"][:NPC]
    return outp


# revision 63
# speedup vs baseline: 1.3389x; 1.0243x over previous
"""ARMA-style GNN message passing on 8 TRN2 NeuronCores.

Reference computation (per layer, 7 layers):
    m   = h @ W                                  [N, CH]
    agg = segment_sum(w[:,None] * m[dst], src)   [N, CH]
    h'  = relu(agg + h @ V + b)
then logits = h @ Wd + bd.

Strategy (graph/data parallel over nodes, ReduceScatter formulation):
  - 8 cores own 1250 nodes each (padded to 1280 = 10 blocks of 128).
  - Edge (s, d) is processed by the core owning d (where the message row
    m[d] is LOCAL).  Each core computes, per layer, a partial aggregation
    table over the full padded-global src space [8*1280, CH]:
        partial_p[s] = sum_{e: dst_e in p, src_e = s} w_e * m_local[dst_e]
    then one ReduceScatter(add) sums the 8 partials and hands every core
    the [1280, CH] slice for its own nodes.  (A ReduceScatter's output is
    1/8 the size of the AllGather the src-side formulation needs, which is
    what the inter-chip collective cost scales with.)
  - Edges are bucketed by global src block (80 buckets of 128 src slots).
    Within a bucket the distinct local dst rows are deduplicated into
    gather tiles of 128 rows; one indirect DMA fetches a tile from the
    local m table in DRAM, and a [128 dst-rows x 128 src-slots] bf16
    "selection" matrix carrying the degree weights w_e scatter-adds it
    into the bucket's PSUM accumulator on PE.
  - Layer 1 scatters x itself (256 wide, half the traffic) and applies W1
    after the ReduceScatter: A@(x W1) == (A@x) W1.
  - h@V + b is precomputed into SBUF while the ReduceScatter is in
    flight; the post-collective epilogue is add + relu + PE-transpose.
  - Final dense layer and output assembly per core; host concatenates.

All matmuls run in bf16 with fp32 PSUM accumulation.
"""
import numpy as np
import ml_dtypes

import concourse.bass as bass
import concourse.tile as tile
import concourse.mybir as mybir
from concourse.vector_clock import ScopedClock
from concourse.bass_utils import run_bass_kernel_spmd
from concourse.masks import make_identity

# ---------------------------------------------------------------- constants
N_NODES = 10000
N_EDGES = 160000
IN_F = 256
CH = 512
N_LABELS = 1440
NCORES = 8
NPC = N_NODES // NCORES      # 1250 nodes per core
P = 128
NBL = 10                     # node blocks per core (10*128 = 1280)
NPAD = NBL * P               # padded nodes per core
NSB = NCORES * NBL           # global src blocks (80)
NGPAD = NCORES * NPAD        # padded global node space (10240)
NLAYERS = 7
KG1 = IN_F // P              # 2 contraction blocks in layer 1
KGC = CH // P                # 4 contraction blocks in layers 2..7
FIN_CHUNK = 480              # 1440 = 3 * 480, fits one PSUM bank in f32

BF = mybir.dt.bfloat16
F32 = mybir.dt.float32
BFNP = ml_dtypes.bfloat16


# ------------------------------------------------------- walrus workarounds
def _patched_drain_and_barrier(self, tick_clock, wait_clock):
    # This walrus build rejects >1-2 sync waits on one TPB_CTRL; put the
    # kernel-tail drain's waits on separate preceding SP nops instead.
    nc = self.nc
    probe = nc.sync.nop(nofuse=True, hint="drain_waits")
    wait_clock.add_sem_waits(probe.ins, ScopedClock({None: tick_clock.global_clock}))
    si = probe.ins.sync_info
    waits = list(si.on_wait) if si is not None else []
    if len(waits) > 1:
        si.on_wait = waits[:1]
        for i in range(1, len(waits)):
            n2 = nc.sync.nop(nofuse=True, hint=f"drain_waits_{i}")
            n2.ins.sync_info = mybir.SyncInfo(on_wait=[waits[i]], on_update=[])
    nc.sync.drain()
    nc.all_engine_barrier()
    assert self.sems is not None
    popped = nc._tile_sem_poison_stack.pop()
    assert popped is self._sem_poison
    nc.clear_and_free_semaphores(list(self.sems.allocated().values()))
    nc.all_engine_barrier()


tile.TileContext._drain_and_barrier = _patched_drain_and_barrier


def _split_excess_waits(nc, limit=1):
    # Same ISA restriction for ordinary instructions: hoist excess sync
    # waits onto injected same-engine nops placed just before.
    for func in nc.m.functions:
        for bb in func.blocks:
            out = []
            for ins in bb.instructions:
                si = ins.sync_info
                if si is not None and si.on_wait and len(si.on_wait) > limit:
                    waits = list(si.on_wait)
                    excess, keep = waits[:-limit], waits[-limit:]
                    for i in range(0, len(excess), limit):
                        out.append(mybir.InstNoOp(
                            name=f"{ins.name}_xw{i}",
                            engine=ins.engine,
                            ins=[], outs=[],
                            sync_info=mybir.SyncInfo(
                                on_wait=excess[i:i + limit], on_update=[]),
                        ))
                    si.on_wait = keep
                out.append(ins)
            bb.instructions[:] = out


# ------------------------------------------------------------- host prep
SUBGROUPS = ((0, 1, 2, 3), (4, 5, 6), (7, 8, 9))   # local src blocks per group
NSG = len(SUBGROUPS)
NGRP = NCORES * NSG           # 24 groups; group g = (core g//NSG, sub g%NSG)
CHUNKS = ((0, 1), (2,))       # subgroup ids per ReduceScatter chunk
CHUNK_R0 = (0, 7)             # first local block of each chunk
CHUNK_NB = (7, 3)             # local blocks per chunk
SUBSZ = tuple(len(s) for s in SUBGROUPS)
SUB_OF = [si for si, s in enumerate(SUBGROUPS) for _ in s]
FIRSTB = tuple(s[0] for s in SUBGROUPS)
SMAX = max(SUBSZ)
EROW = CHUNK_NB[0] * P        # gather rows below this are written early
MID_BACK = 6                  # hv insert point: groups before chunk-1 end


def _prep_edges(src, dst):
    """Route each edge to the core owning dst.  The 80 global src blocks
    form 24 groups (per src-owning core: blocks 0-3, 4-6, 7-9; the 7/3
    block split matches the two ReduceScatter chunks).  Per group the
    distinct local dst rows are deduplicated into gather tiles of 128 and
    gathered once; each tile then feeds one sel-matmul per block in the
    group.  Returns (ntiles, idx_tabs, sel_tabs): ntiles[g] = tile count
    of group g (max over cores, same program on all cores); per core an
    idx table [128, NT] of local m-table rows and a sel table of bf16
    degree weights, column blocks ordered (group, tile, block-in-group)."""
    src = np.asarray(src).astype(np.int64)
    dst = np.asarray(dst).astype(np.int64)
    deg_out = np.maximum(np.bincount(src, minlength=N_NODES), 1.0).astype(np.float32)
    deg_in = np.maximum(np.bincount(dst, minlength=N_NODES), 1.0).astype(np.float32)
    w = (1.0 / np.sqrt(deg_out[src] * deg_in[dst])).astype(np.float32)

    core = dst // NPC
    ldst = dst - core * NPC
    psrc = (src // NPC) * NPAD + (src % NPC)   # padded-global src row
    sb = psrc // P
    cg_e = sb // NBL
    lb_e = sb - cg_e * NBL
    si_e = np.asarray(SUB_OF)[lb_e]
    grp = cg_e * NSG + si_e
    j_e = lb_e - np.asarray(FIRSTB)[si_e]      # block index within group
    slot = psrc - sb * P

    order = np.lexsort((ldst, grp, core))
    c_s, g_s = core[order], grp[order]
    ld_s, j_s, sl_s, w_s = ldst[order], j_e[order], slot[order], w[order]
    key = c_s * NGRP + g_s
    starts = np.searchsorted(key, np.arange(NCORES * NGRP))
    ends = np.searchsorted(key, np.arange(NCORES * NGRP), side="right")

    # Tiles keep the group's sorted distinct rows; a tile whose rows are
    # all < EROW ("early") is gathered through a prefix-slice AP of the m
    # table, so it only depends on the early node blocks, which the
    # interleaved epilogue writes before the trailing collective ends.
    uniq_store = {}
    ntiles = np.ones(NGRP, np.int64)
    for p in range(NCORES):
        for g in range(NGRP):
            s0, s1 = starts[p * NGRP + g], ends[p * NGRP + g]
            if s1 > s0:
                uniq, inv = np.unique(ld_s[s0:s1], return_inverse=True)
            else:
                uniq, inv = np.zeros(1, np.int64), np.zeros(0, np.int64)
            uniq_store[p, g] = (uniq, inv, s0, s1)
            ntiles[g] = max(ntiles[g], -(-len(uniq) // P))
    subsz = np.asarray([SUBSZ[g % NSG] for g in range(NGRP)])
    colof = np.zeros(NGRP, np.int64)           # tile-column offsets (idx)
    colof[1:] = np.cumsum(ntiles)[:-1]
    czof = np.zeros(NGRP, np.int64)            # 128-col-block offsets (sel)
    czof[1:] = np.cumsum(ntiles * subsz)[:-1]
    nt = int(ntiles.sum())
    ncb = int((ntiles * subsz).sum())

    # early-tile count per group: tiles all of whose lanes (on EVERY core)
    # index rows < EROW
    ntA = ntiles.copy()
    for p in range(NCORES):
        for g in range(NGRP):
            uniq, inv, s0, s1 = uniq_store[p, g]
            na_rows = int((uniq < EROW).sum())
            ea = ntiles[g] if na_rows == len(uniq) else na_rows // P
            ntA[g] = min(ntA[g], ea)

    idx_tabs, sel_tabs = [], []
    for p in range(NCORES):
        idx_t = np.zeros((P, nt), np.int32)
        sel_t = np.zeros((P, ncb * P), np.float32)
        for g in range(NGRP):
            uniq, inv, s0, s1 = uniq_store[p, g]
            lanes_u = np.arange(len(uniq))
            idx_t[lanes_u % P, colof[g] + lanes_u // P] = uniq
            if s1 > s0:
                tile_e = inv // P
                lane_e = inv - tile_e * P
                cb = czof[g] + tile_e * subsz[g] + j_s[s0:s1]
                np.add.at(sel_t, (lane_e, cb * P + sl_s[s0:s1]), w_s[s0:s1])
        idx_tabs.append(idx_t)
        sel_tabs.append(sel_t.astype(BFNP))
    return [(int(n), int(a)) for n, a in zip(ntiles, ntA)], idx_tabs, sel_tabs


def _pack_lhsT(xT, kg):
    """[kg*128, NPAD] -> [128, kg*NPAD] (partition-major kg blocks)."""
    return np.ascontiguousarray(
        xT.reshape(kg, P, NPAD).transpose(1, 0, 2).reshape(P, kg * NPAD))


def _pack_rhs(Wm, kg, n):
    """[kg*128, n] -> [128, kg*n]."""
    return np.ascontiguousarray(
        Wm.reshape(kg, P, n).transpose(1, 0, 2).reshape(P, kg * n))


# ------------------------------------------------------------- device build
def _build(ntiles, repeat=1):
    ntiles_a = np.asarray([t[0] for t in ntiles])
    ntA = np.asarray([t[1] for t in ntiles])
    nt = int(ntiles_a.sum())
    ntmax = int(ntiles_a.max())
    subsz = np.asarray([SUBSZ[g % NSG] for g in range(NGRP)])
    colof = np.zeros(NGRP, np.int64)
    colof[1:] = np.cumsum(ntiles_a)[:-1]
    czof = np.zeros(NGRP, np.int64)
    czof[1:] = np.cumsum(ntiles_a * subsz)[:-1]
    ncb = int((ntiles_a * subsz).sum())
    nc = bass.Bass("TRN2", target_bir_lowering=False, debug=False,
                   num_devices=NCORES)

    def din(name, shape, dt):
        return nc.dram_tensor(name, shape, dt, kind="ExternalInput").ap()

    xT = din("xT", [P, KG1 * NPAD], BF)
    xrows = din("xrows", [NPAD, IN_F], BF)
    idx = din("idx", [P, nt], mybir.dt.int32)
    sel = din("sel", [P, ncb * P], BF)
    w1 = din("w1", [P, KG1 * CH], BF)
    v1 = din("v1", [P, KG1 * CH], BF)
    wk = din("wk", [P, 6 * KGC * CH], BF)
    vk = din("vk", [P, 6 * KGC * CH], BF)
    wd = din("wd", [P, KGC * N_LABELS], BF)
    ball = din("ball", [P, NLAYERS * CH], F32)
    bdr = din("bdr", [P, N_LABELS], F32)
    out = nc.dram_tensor("out", [NPAD, N_LABELS], F32, kind="ExternalOutput").ap()

    with tile.TileContext(nc) as tc:
        with (
            tc.tile_pool(name="const", bufs=1) as cp,
            tc.tile_pool(name="ht", bufs=2) as htp,
            tc.tile_pool(name="mout", bufs=2) as mp,
            tc.tile_pool(name="selg", bufs=3) as selp,
            tc.tile_pool(name="msg", bufs=30) as msgp,
            tc.tile_pool(name="pw", bufs=4) as pwp,
            tc.tile_pool(name="agg", bufs=4) as aggp,
            tc.tile_pool(name="hv", bufs=11) as hvp,
            tc.tile_pool(name="ep", bufs=3) as epp,
            tc.tile_pool(name="ut", bufs=2) as utp,
            tc.tile_pool(name="hact", bufs=2) as hp,
            tc.tile_pool(name="outs", bufs=2) as op,
            tc.tile_pool(name="psm", bufs=2, space="PSUM") as psm,
            tc.tile_pool(name="psagg", bufs=4, space="PSUM") as psagg,
            tc.tile_pool(name="pstr", bufs=2, space="PSUM") as pstr,
            tc.tile_pool(name="dram", bufs=1, space="DRAM") as dram,
        ):
            # ---- constants to SBUF, roughly in first-use order
            idx_t = cp.tile([P, nt], mybir.dt.int32)
            nc.sync.dma_start(idx_t[:], idx[:])
            w1_t = cp.tile([P, KG1 * CH], BF)
            nc.sync.dma_start(w1_t[:], w1[:])
            v1_t = cp.tile([P, KG1 * CH], BF)
            nc.sync.dma_start(v1_t[:], v1[:])
            xT_t = cp.tile([P, KG1 * NPAD], BF)
            nc.sync.dma_start(xT_t[:], xT[:])
            ball_t = cp.tile([P, NLAYERS * CH], F32)
            nc.sync.dma_start(ball_t[:], ball[:])
            wk_t = cp.tile([P, 6 * KGC * CH], BF)
            nc.sync.dma_start(wk_t[:], wk[:])
            vk_t = cp.tile([P, 6 * KGC * CH], BF)
            wd_t = cp.tile([P, KGC * N_LABELS], BF)
            bdr_t = cp.tile([P, N_LABELS], F32)
            ident = cp.tile([P, P], BF)
            make_identity(nc, ident[:])

            def scatter_phase(width, gsrc_full, gsrc_pref, rs_out, ptag,
                              mid_cb=None):
                """Grouped scatter.  Per chunk: first queue EVERY group's
                early-tile gathers (rows < EROW, via the prefix-slice AP,
                so they depend only on the early m-table blocks and can run
                under the previous layer's trailing collective), then per
                group stream the sel columns, gather the late tiles, and
                scatter-add into each block's accumulator.  After a chunk's
                partial writes, launch its ReduceScatter chunk.  mid_cb
                (the chunk-0 epilogue) is emitted near the end of chunk 1
                so its early m-table writes land before that collective
                finishes."""
                LOOKAHEAD = 6
                for ci, subids in enumerate(CHUNKS):
                    partial = dram.tile([NCORES, CHUNK_NB[ci] * P, width], BF,
                                        tag=f"{ptag}c{ci}")
                    seq = [cg * NSG + si
                           for cg in range(NCORES) for si in subids]
                    # fewest-late-tiles first: extends the early runway at
                    # the layer boundary before the first late-gather wait
                    seq.sort(key=lambda g: ntiles_a[g] - ntA[g])
                    gmsgs = {}

                    def emit_early(g):
                        co = int(colof[g])
                        msgs = []
                        for t in range(int(ntA[g])):
                            msg = msgp.tile([P, CH], BF, tag="msg")
                            nc.gpsimd.indirect_dma_start(
                                out=msg[:, :width], out_offset=None,
                                in_=gsrc_pref,
                                in_offset=bass.IndirectOffsetOnAxis(
                                    ap=idx_t[:, co + t:co + t + 1], axis=0))
                            msgs.append(msg)
                        gmsgs[g] = msgs

                    for k in range(min(LOOKAHEAD, len(seq))):
                        emit_early(seq[k])
                    for k, g in enumerate(seq):
                        if (ci == len(CHUNKS) - 1
                                and k == len(seq) - 1 - MID_BACK
                                and mid_cb is not None):
                            mid_cb()
                        if True:
                            cg, si = g // NSG, g % NSG
                            co, cz = int(colof[g]), int(czof[g])
                            ntg, S = int(ntiles_a[g]), SUBSZ[si]
                            selg = selp.tile([P, ntmax * SMAX * P], BF,
                                             tag="selg")
                            nc.scalar.dma_start(
                                selg[:, :ntg * S * P],
                                sel[:, cz * P:(cz + ntg * S) * P])
                            if k + LOOKAHEAD < len(seq):
                                emit_early(seq[k + LOOKAHEAD])
                            msgs = gmsgs[g]
                            for t in range(int(ntA[g]), ntg):
                                msg = msgp.tile([P, CH], BF, tag="msg")
                                nc.gpsimd.indirect_dma_start(
                                    out=msg[:, :width], out_offset=None,
                                    in_=gsrc_full,
                                    in_offset=bass.IndirectOffsetOnAxis(
                                        ap=idx_t[:, co + t:co + t + 1], axis=0))
                                msgs.append(msg)
                            row0 = (FIRSTB[si] - CHUNK_R0[ci]) * P
                            for j in range(S):
                                acc = psagg.tile([P, CH], F32, tag="agg")
                                for t in range(ntg):
                                    nc.tensor.matmul(
                                        acc[:, :width],
                                        selg[:, (t * S + j) * P:(t * S + j + 1) * P],
                                        msgs[t][:, :width],
                                        start=(t == 0), stop=(t == ntg - 1))
                                pwt = pwp.tile([P, CH], BF, tag="pw")
                                nc.vector.tensor_copy(pwt[:, :width],
                                                      acc[:, :width])
                                nc.sync.dma_start(
                                    partial[cg, row0 + j * P:
                                            row0 + (j + 1) * P, :],
                                    pwt[:, :width])
                    nc.gpsimd.collective_compute(
                        "ReduceScatter", mybir.AluOpType.add,
                        replica_groups=[list(range(NCORES))],
                        ins=[partial[:].opt()],
                        outs=[rs_out[CHUNK_R0[ci] * P:
                                     (CHUNK_R0[ci] + CHUNK_NB[ci]) * P,
                                     :].opt()])

            for rep in range(repeat):
                # ================= layer 1 (x-form) =================
                # hv1 = x @ V1 + b1, precomputed to SBUF (overlaps scatter/RS)
                hv_sb = []
                for b in range(NBL):
                    hps = psm.tile([P, CH], F32, tag="m")
                    for g in range(KG1):
                        nc.tensor.matmul(
                            hps[:],
                            xT_t[:, g * NPAD + b * P:g * NPAD + (b + 1) * P],
                            v1_t[:, g * CH:(g + 1) * CH],
                            start=(g == 0), stop=(g == KG1 - 1))
                    hvb = hvp.tile([P, CH], BF, tag="hv")
                    nc.vector.tensor_add(hvb[:], hps[:], ball_t[:, 0:CH])
                    hv_sb.append(hvb)

                rs1 = dram.tile([NPAD, IN_F], BF, tag=f"rs0_{rep}")

                def emit_m(hT, b, l_next, mtab_next):
                    """m = h @ W for node block b of the next conv layer,
                    written into its m-table half (A for blocks 0-6, B for
                    7-9, so the next layer's early gathers can start while
                    the trailing ReduceScatter chunk still runs)."""
                    wt = wk_t[:, (l_next - 1) * KGC * CH:l_next * KGC * CH]
                    mps = psm.tile([P, CH], F32, tag="m")
                    for g in range(KGC):
                        nc.tensor.matmul(
                            mps[:],
                            hT[:, g * NPAD + b * P:g * NPAD + (b + 1) * P],
                            wt[:, g * CH:(g + 1) * CH],
                            start=(g == 0), stop=(g == KGC - 1))
                    m_bf = mp.tile([P, CH], BF, tag="mbf")
                    nc.scalar.activation(
                        m_bf[:], mps[:], mybir.ActivationFunctionType.Copy)
                    nc.scalar.dma_start(
                        mtab_next[b * P:(b + 1) * P, :], m_bf[:])

                def emit_final(hT, b):
                    o_sb = op.tile([P, N_LABELS], F32, tag="o")
                    fps = []
                    for c in range(3):
                        fin_ps = psagg.tile([P, FIN_CHUNK], F32, tag="agg")
                        fps.append(fin_ps)
                    for g in range(KGC):
                        for c in range(3):
                            nc.tensor.matmul(
                                fps[c][:],
                                hT[:, g * NPAD + b * P:g * NPAD + (b + 1) * P],
                                wd_t[:, g * N_LABELS + c * FIN_CHUNK:
                                     g * N_LABELS + (c + 1) * FIN_CHUNK],
                                start=(g == 0), stop=(g == KGC - 1))
                    for c in range(3):
                        sl = slice(c * FIN_CHUNK, (c + 1) * FIN_CHUNK)
                        nc.vector.tensor_add(fps[c][:], fps[c][:], bdr_t[:, sl])
                        nc.scalar.activation(
                            o_sb[:, sl], fps[c][:],
                            mybir.ActivationFunctionType.Copy)
                    if rep == repeat - 1:
                        nc.sync.dma_start(out[b * P:(b + 1) * P, :], o_sb[:])

                def alloc_mtab(l):
                    mtab_n = dram.tile([NPAD, CH], BF, tag=f"mt{l % 2}")
                    return mtab_n

                hT_cur = htp.tile([P, KGC * NPAD], BF, tag="hT")
                mtab_next = alloc_mtab(1)

                def epi1_block(b):
                    u_sb = aggp.tile([P, CH], BF, tag="aggs")
                    nc.scalar.dma_start(u_sb[:, :IN_F],
                                        rs1[b * P:(b + 1) * P, :])
                    utt = utp.tile([P, IN_F], BF, tag="ut")
                    for g in range(KG1):
                        tr = pstr.tile([P, P], BF, tag="tr")
                        nc.tensor.transpose(
                            tr[:], u_sb[:, g * P:(g + 1) * P], ident[:])
                        nc.vector.tensor_copy(utt[:, g * P:(g + 1) * P], tr[:])
                    ups = psm.tile([P, CH], F32, tag="m")
                    for g in range(KG1):
                        nc.tensor.matmul(
                            ups[:], utt[:, g * P:(g + 1) * P],
                            w1_t[:, g * CH:(g + 1) * CH],
                            start=(g == 0), stop=(g == KG1 - 1))
                    ep = epp.tile([P, CH], BF, tag="ep")
                    nc.vector.tensor_add(ep[:], ups[:], hv_sb[b][:])
                    h_bf = hp.tile([P, CH], BF, tag="h")
                    nc.scalar.activation(
                        h_bf[:], ep[:], mybir.ActivationFunctionType.Relu)
                    for cg in range(KGC):
                        tr2 = pstr.tile([P, P], BF, tag="tr")
                        nc.tensor.transpose(
                            tr2[:], h_bf[:, cg * P:(cg + 1) * P], ident[:])
                        nc.vector.tensor_copy(
                            hT_cur[:, cg * NPAD + b * P:cg * NPAD + (b + 1) * P],
                            tr2[:])
                    emit_m(hT_cur, b, 1, mtab_next)

                scatter_phase(IN_F, xrows[:], xrows[:EROW], rs1, f"ptx{rep}")
                if rep == 0:
                    nc.sync.dma_start(vk_t[:], vk[:])
                    nc.sync.dma_start(wd_t[:], wd[:])
                    nc.sync.dma_start(bdr_t[:], bdr[:])
                for b in range(NBL):
                    epi1_block(b)

                # ================= layers 2..7 (m-form) =================
                for l in range(1, NLAYERS):
                    vt = vk_t[:, (l - 1) * KGC * CH:l * KGC * CH]
                    mtab = mtab_next
                    rs = dram.tile([NPAD, CH], BF, tag=f"rs{l}_{rep}")
                    if l < NLAYERS - 1:
                        mtab_next = alloc_mtab(l + 1)
                    hT_next = htp.tile([P, KGC * NPAD], BF, tag="hT")

                    # epilogue: per block hv, add+relu, transposes, then the
                    # next layer's m (or the final dense) for that block;
                    # chunk-0 blocks interleave with the scatter tail so
                    # m-table A lands before the trailing collective
                    hvbs = {}

                    def hv_block(b, l=l, vt=vt, hT_cur=hT_cur):
                        hps = psm.tile([P, CH], F32, tag="m")
                        for g in range(KGC):
                            nc.tensor.matmul(
                                hps[:],
                                hT_cur[:, g * NPAD + b * P:g * NPAD + (b + 1) * P],
                                vt[:, g * CH:(g + 1) * CH],
                                start=(g == 0), stop=(g == KGC - 1))
                        hvb = hvp.tile([P, CH], BF, tag="hv")
                        nc.vector.tensor_add(
                            hvb[:], hps[:], ball_t[:, l * CH:(l + 1) * CH])
                        hvbs[b] = hvb

                    def epi_rest(b, l=l, rs=rs, hT_next=hT_next,
                                 mtab_next=mtab_next):
                        agg_sb = aggp.tile([P, CH], BF, tag="aggs")
                        nc.scalar.dma_start(agg_sb[:], rs[b * P:(b + 1) * P, :])
                        ep = epp.tile([P, CH], BF, tag="ep")
                        nc.vector.tensor_add(ep[:], agg_sb[:], hvbs[b][:])
                        h_bf = hp.tile([P, CH], BF, tag="h")
                        nc.scalar.activation(
                            h_bf[:], ep[:], mybir.ActivationFunctionType.Relu)
                        for cg in range(KGC):
                            tr2 = pstr.tile([P, P], BF, tag="tr")
                            nc.tensor.transpose(
                                tr2[:], h_bf[:, cg * P:(cg + 1) * P], ident[:])
                            nc.vector.tensor_copy(
                                hT_next[:, cg * NPAD + b * P:cg * NPAD + (b + 1) * P],
                                tr2[:])
                        if l < NLAYERS - 1:
                            emit_m(hT_next, b, l + 1, mtab_next)
                        else:
                            emit_final(hT_next, b)

                    scatter_phase(CH, mtab[:], mtab[:EROW], rs,
                                  f"pt{l % 2}_{rep}",
                                  mid_cb=lambda hv_block=hv_block:
                                      [hv_block(b) for b in range(NBL)])
                    for b in range(NBL):
                        epi_rest(b)
                    hT_cur = hT_next

    _split_excess_waits(nc)
    return nc


# ------------------------------------------------------------- entry point
def kernel(x, src, dst, W1, V1, b1, Wk, Vk, bk, Wd, bd, _repeat=1, _nc_cache={}):
    x = np.asarray(x, np.float32)
    ntiles, idx_tabs, sel_tabs = _prep_edges(src, dst)

    key = (tuple(ntiles), _repeat)
    if key not in _nc_cache:
        _nc_cache[key] = _build(ntiles, repeat=_repeat)
    nc = _nc_cache[key]

    # weights (replicated, host-packed)
    w1p = _pack_rhs(np.asarray(W1, np.float32), KG1, CH).astype(BFNP)
    v1p = _pack_rhs(np.asarray(V1, np.float32), KG1, CH).astype(BFNP)
    wkp = np.concatenate(
        [_pack_rhs(np.asarray(Wk[i], np.float32), KGC, CH) for i in range(6)],
        axis=1).astype(BFNP)
    vkp = np.concatenate(
        [_pack_rhs(np.asarray(Vk[i], np.float32), KGC, CH) for i in range(6)],
        axis=1).astype(BFNP)
    wdp = _pack_rhs(np.asarray(Wd, np.float32), KGC, N_LABELS).astype(BFNP)
    ballv = np.concatenate(
        [np.asarray(b1, np.float32)] + [np.asarray(bk[i], np.float32)
                                        for i in range(6)])
    ballp = np.broadcast_to(ballv, (P, NLAYERS * CH)).copy()
    bdp = np.broadcast_to(np.asarray(bd, np.float32), (P, N_LABELS)).copy()

    in_maps = []
    for p in range(NCORES):
        xp = np.zeros((NPAD, IN_F), np.float32)
        xp[:NPC] = x[p * NPC:(p + 1) * NPC]
        xTp = _pack_lhsT(np.ascontiguousarray(xp.T), KG1).astype(BFNP)
        xbf = xp.astype(BFNP)
        in_maps.append({
            "xT": xTp, "xrows": xbf,
            "idx": idx_tabs[p], "sel": sel_tabs[p],
            "w1": w1p, "v1": v1p, "wk": wkp, "vk": vkp, "wd": wdp,
            "ball": ballp, "bdr": bdp,
        })

    res = run_bass_kernel_spmd(nc, in_maps, core_ids=list(range(NCORES)))
    outp = np.empty((N_NODES, N_LABELS), np.float32)
    for p in range(NCORES):
        outp[p * NPC:(p + 1) * NPC] = res.results[p]["out"][:NPC]
    return outp


# revision 66
# speedup vs baseline: 1.3488x; 1.0074x over previous
"""ARMA-style GNN message passing on 8 TRN2 NeuronCores.

Reference computation (per layer, 7 layers):
    m   = h @ W                                  [N, CH]
    agg = segment_sum(w[:,None] * m[dst], src)   [N, CH]
    h'  = relu(agg + h @ V + b)
then logits = h @ Wd + bd.

Strategy (graph/data parallel over nodes, ReduceScatter formulation):
  - 8 cores own 1250 nodes each (padded to 1280 = 10 blocks of 128).
  - Edge (s, d) is processed by the core owning d (where the message row
    m[d] is LOCAL).  Each core computes, per layer, a partial aggregation
    table over the full padded-global src space [8*1280, CH]:
        partial_p[s] = sum_{e: dst_e in p, src_e = s} w_e * m_local[dst_e]
    then ReduceScatter(add) sums the 8 partials and hands every core the
    [1280, CH] slice for its own nodes.  (A ReduceScatter's output is 1/8
    the size of the AllGather the src-side formulation needs, which is
    what the inter-chip collective cost scales with.)  The collective is
    split into two row chunks (node blocks 0-6 / 7-9) so most of it
    overlaps compute.
  - The 80 global src blocks form 24 groups (per src core: blocks 0-3,
    4-6, 7-9).  Per group, the distinct local dst rows are deduplicated
    (each indirect-DMA descriptor-gen on GPSIMD costs ~1us regardless of
    size, so gather instructions are the scarce resource) into gather
    tiles of 128 rows, fetched once and reused by one sel-matmul per
    block in the group: a [128 dst-rows x 128 src-slots] bf16 "selection"
    matrix carrying the degree weights w_e scatter-adds the tile into the
    block's PSUM accumulator on PE.  Sel matrices stream from DRAM per
    group on the Activation HWDGE queue.
  - Cross-layer pipelining: tiles whose rows all fall in node blocks 0-6
    gather through a prefix-slice AP of the m table (region-granular
    dependency tracking), so they start while the previous layer's
    trailing collective chunk and late epilogue still run; a sliding
    lookahead keeps them ahead of late-tile gathers in the Pool queue,
    and groups with fewest late tiles are processed first.
  - h@V + b matmuls are interleaved near the end of the scatter emission
    (no collective dependency); the collective-dependent epilogue part
    (add + relu + PE-transpose) plus the NEXT layer's m = h @ W for each
    node block run after, overlapping the trailing collective chunk.
  - Layer 1 scatters x itself (256 wide, half the traffic) and applies W1
    after the ReduceScatter: A@(x W1) == (A@x) W1.
  - Final dense layer is folded into layer 7's per-block epilogue.

All matmuls run in bf16 with fp32 PSUM accumulation.
"""
import numpy as np
import ml_dtypes

import concourse.bass as bass
import concourse.tile as tile
import concourse.mybir as mybir
from concourse.vector_clock import ScopedClock
from concourse.bass_utils import run_bass_kernel_spmd
from concourse.masks import make_identity

# ---------------------------------------------------------------- constants
N_NODES = 10000
N_EDGES = 160000
IN_F = 256
CH = 512
N_LABELS = 1440
NCORES = 8
NPC = N_NODES // NCORES      # 1250 nodes per core
P = 128
NBL = 10                     # node blocks per core (10*128 = 1280)
NPAD = NBL * P               # padded nodes per core
NSB = NCORES * NBL           # global src blocks (80)
NGPAD = NCORES * NPAD        # padded global node space (10240)
NLAYERS = 7
KG1 = IN_F // P              # 2 contraction blocks in layer 1
KGC = CH // P                # 4 contraction blocks in layers 2..7
FIN_CHUNK = 480              # 1440 = 3 * 480, fits one PSUM bank in f32

BF = mybir.dt.bfloat16
F32 = mybir.dt.float32
BFNP = ml_dtypes.bfloat16


# ------------------------------------------------------- walrus workarounds
def _patched_drain_and_barrier(self, tick_clock, wait_clock):
    # This walrus build rejects >1-2 sync waits on one TPB_CTRL; put the
    # kernel-tail drain's waits on separate preceding SP nops instead.
    nc = self.nc
    probe = nc.sync.nop(nofuse=True, hint="drain_waits")
    wait_clock.add_sem_waits(probe.ins, ScopedClock({None: tick_clock.global_clock}))
    si = probe.ins.sync_info
    waits = list(si.on_wait) if si is not None else []
    if len(waits) > 1:
        si.on_wait = waits[:1]
        for i in range(1, len(waits)):
            n2 = nc.sync.nop(nofuse=True, hint=f"drain_waits_{i}")
            n2.ins.sync_info = mybir.SyncInfo(on_wait=[waits[i]], on_update=[])
    nc.sync.drain()
    nc.all_engine_barrier()
    assert self.sems is not None
    popped = nc._tile_sem_poison_stack.pop()
    assert popped is self._sem_poison
    nc.clear_and_free_semaphores(list(self.sems.allocated().values()))
    nc.all_engine_barrier()


tile.TileContext._drain_and_barrier = _patched_drain_and_barrier


def _split_excess_waits(nc, limit=1):
    # Same ISA restriction for ordinary instructions: hoist excess sync
    # waits onto injected same-engine nops placed just before.
    for func in nc.m.functions:
        for bb in func.blocks:
            out = []
            for ins in bb.instructions:
                si = ins.sync_info
                if si is not None and si.on_wait and len(si.on_wait) > limit:
                    waits = list(si.on_wait)
                    excess, keep = waits[:-limit], waits[-limit:]
                    for i in range(0, len(excess), limit):
                        out.append(mybir.InstNoOp(
                            name=f"{ins.name}_xw{i}",
                            engine=ins.engine,
                            ins=[], outs=[],
                            sync_info=mybir.SyncInfo(
                                on_wait=excess[i:i + limit], on_update=[]),
                        ))
                    si.on_wait = keep
                out.append(ins)
            bb.instructions[:] = out


# ------------------------------------------------------------- host prep
SUBGROUPS = ((0, 1, 2, 3), (4, 5, 6), (7, 8, 9))   # local src blocks per group
NSG = len(SUBGROUPS)
NGRP = NCORES * NSG           # 24 groups; group g = (core g//NSG, sub g%NSG)
CHUNKS = ((0, 1), (2,))       # subgroup ids per ReduceScatter chunk
CHUNK_R0 = (0, 7)             # first local block of each chunk
CHUNK_NB = (7, 3)             # local blocks per chunk
SUBSZ = tuple(len(s) for s in SUBGROUPS)
SUB_OF = [si for si, s in enumerate(SUBGROUPS) for _ in s]
FIRSTB = tuple(s[0] for s in SUBGROUPS)
SMAX = max(SUBSZ)
EROW = CHUNK_NB[0] * P        # gather rows below this are written early
MID_BACK = 6                  # hv insert point: groups before chunk-1 end


def _prep_edges(src, dst):
    """Route each edge to the core owning dst.  The 80 global src blocks
    form 24 groups (per src-owning core: blocks 0-3, 4-6, 7-9; the 7/3
    block split matches the two ReduceScatter chunks).  Per group the
    distinct local dst rows are deduplicated into gather tiles of 128 and
    gathered once; each tile then feeds one sel-matmul per block in the
    group.  Returns (ntiles, idx_tabs, sel_tabs): ntiles[g] = tile count
    of group g (max over cores, same program on all cores); per core an
    idx table [128, NT] of local m-table rows and a sel table of bf16
    degree weights, column blocks ordered (group, tile, block-in-group)."""
    src = np.asarray(src).astype(np.int64)
    dst = np.asarray(dst).astype(np.int64)
    deg_out = np.maximum(np.bincount(src, minlength=N_NODES), 1.0).astype(np.float32)
    deg_in = np.maximum(np.bincount(dst, minlength=N_NODES), 1.0).astype(np.float32)
    w = (1.0 / np.sqrt(deg_out[src] * deg_in[dst])).astype(np.float32)

    core = dst // NPC
    ldst = dst - core * NPC
    psrc = (src // NPC) * NPAD + (src % NPC)   # padded-global src row
    sb = psrc // P
    cg_e = sb // NBL
    lb_e = sb - cg_e * NBL
    si_e = np.asarray(SUB_OF)[lb_e]
    grp = cg_e * NSG + si_e
    j_e = lb_e - np.asarray(FIRSTB)[si_e]      # block index within group
    slot = psrc - sb * P

    order = np.lexsort((ldst, grp, core))
    c_s, g_s = core[order], grp[order]
    ld_s, j_s, sl_s, w_s = ldst[order], j_e[order], slot[order], w[order]
    key = c_s * NGRP + g_s
    starts = np.searchsorted(key, np.arange(NCORES * NGRP))
    ends = np.searchsorted(key, np.arange(NCORES * NGRP), side="right")

    # Tiles keep the group's sorted distinct rows; a tile whose rows are
    # all < EROW ("early") is gathered through a prefix-slice AP of the m
    # table, so it only depends on the early node blocks, which the
    # interleaved epilogue writes before the trailing collective ends.
    uniq_store = {}
    ntiles = np.ones(NGRP, np.int64)
    for p in range(NCORES):
        for g in range(NGRP):
            s0, s1 = starts[p * NGRP + g], ends[p * NGRP + g]
            if s1 > s0:
                uniq, inv = np.unique(ld_s[s0:s1], return_inverse=True)
            else:
                uniq, inv = np.zeros(1, np.int64), np.zeros(0, np.int64)
            uniq_store[p, g] = (uniq, inv, s0, s1)
            ntiles[g] = max(ntiles[g], -(-len(uniq) // P))
    subsz = np.asarray([SUBSZ[g % NSG] for g in range(NGRP)])
    colof = np.zeros(NGRP, np.int64)           # tile-column offsets (idx)
    colof[1:] = np.cumsum(ntiles)[:-1]
    czof = np.zeros(NGRP, np.int64)            # 128-col-block offsets (sel)
    czof[1:] = np.cumsum(ntiles * subsz)[:-1]
    nt = int(ntiles.sum())
    ncb = int((ntiles * subsz).sum())

    # early-tile count per group: tiles all of whose lanes (on EVERY core)
    # index rows < EROW
    ntA = ntiles.copy()
    for p in range(NCORES):
        for g in range(NGRP):
            uniq, inv, s0, s1 = uniq_store[p, g]
            na_rows = int((uniq < EROW).sum())
            ea = ntiles[g] if na_rows == len(uniq) else na_rows // P
            ntA[g] = min(ntA[g], ea)

    idx_tabs, sel_tabs = [], []
    for p in range(NCORES):
        idx_t = np.zeros((P, nt), np.int32)
        sel_t = np.zeros((P, ncb * P), np.float32)
        for g in range(NGRP):
            uniq, inv, s0, s1 = uniq_store[p, g]
            lanes_u = np.arange(len(uniq))
            idx_t[lanes_u % P, colof[g] + lanes_u // P] = uniq
            if s1 > s0:
                tile_e = inv // P
                lane_e = inv - tile_e * P
                cb = czof[g] + tile_e * subsz[g] + j_s[s0:s1]
                np.add.at(sel_t, (lane_e, cb * P + sl_s[s0:s1]), w_s[s0:s1])
        idx_tabs.append(idx_t)
        sel_tabs.append(sel_t.astype(BFNP))
    return [(int(n), int(a)) for n, a in zip(ntiles, ntA)], idx_tabs, sel_tabs


def _pack_lhsT(xT, kg):
    """[kg*128, NPAD] -> [128, kg*NPAD] (partition-major kg blocks)."""
    return np.ascontiguousarray(
        xT.reshape(kg, P, NPAD).transpose(1, 0, 2).reshape(P, kg * NPAD))


def _pack_rhs(Wm, kg, n):
    """[kg*128, n] -> [128, kg*n]."""
    return np.ascontiguousarray(
        Wm.reshape(kg, P, n).transpose(1, 0, 2).reshape(P, kg * n))


# ------------------------------------------------------------- device build
def _build(ntiles, repeat=1):
    ntiles_a = np.asarray([t[0] for t in ntiles])
    ntA = np.asarray([t[1] for t in ntiles])
    nt = int(ntiles_a.sum())
    ntmax = int(ntiles_a.max())
    subsz = np.asarray([SUBSZ[g % NSG] for g in range(NGRP)])
    colof = np.zeros(NGRP, np.int64)
    colof[1:] = np.cumsum(ntiles_a)[:-1]
    czof = np.zeros(NGRP, np.int64)
    czof[1:] = np.cumsum(ntiles_a * subsz)[:-1]
    ncb = int((ntiles_a * subsz).sum())
    nc = bass.Bass("TRN2", target_bir_lowering=False, debug=False,
                   num_devices=NCORES)

    def din(name, shape, dt):
        return nc.dram_tensor(name, shape, dt, kind="ExternalInput").ap()

    xT = din("xT", [P, KG1 * NPAD], BF)
    xrows = din("xrows", [NPAD, IN_F], BF)
    idx = din("idx", [P, nt], mybir.dt.int32)
    sel = din("sel", [P, ncb * P], BF)
    w1 = din("w1", [P, KG1 * CH], BF)
    v1 = din("v1", [P, KG1 * CH], BF)
    wk = din("wk", [P, 6 * KGC * CH], BF)
    vk = din("vk", [P, 6 * KGC * CH], BF)
    wd = din("wd", [P, KGC * N_LABELS], BF)
    ball = din("ball", [P, NLAYERS * CH], F32)
    bdr = din("bdr", [P, N_LABELS], F32)
    out = nc.dram_tensor("out", [NPAD, N_LABELS], F32, kind="ExternalOutput").ap()

    with tile.TileContext(nc) as tc:
        with (
            tc.tile_pool(name="const", bufs=1) as cp,
            tc.tile_pool(name="ht", bufs=2) as htp,
            tc.tile_pool(name="mout", bufs=2) as mp,
            tc.tile_pool(name="selg", bufs=3) as selp,
            tc.tile_pool(name="msg", bufs=30) as msgp,
            tc.tile_pool(name="pw", bufs=4) as pwp,
            tc.tile_pool(name="agg", bufs=4) as aggp,
            tc.tile_pool(name="hv", bufs=11) as hvp,
            tc.tile_pool(name="ep", bufs=3) as epp,
            tc.tile_pool(name="ut", bufs=2) as utp,
            tc.tile_pool(name="hact", bufs=2) as hp,
            tc.tile_pool(name="outs", bufs=2) as op,
            tc.tile_pool(name="psm", bufs=2, space="PSUM") as psm,
            tc.tile_pool(name="psagg", bufs=4, space="PSUM") as psagg,
            tc.tile_pool(name="pstr", bufs=2, space="PSUM") as pstr,
            tc.tile_pool(name="dram", bufs=1, space="DRAM") as dram,
        ):
            # ---- constants to SBUF, roughly in first-use order
            idx_t = cp.tile([P, nt], mybir.dt.int32)
            nc.sync.dma_start(idx_t[:], idx[:])
            w1_t = cp.tile([P, KG1 * CH], BF)
            nc.sync.dma_start(w1_t[:], w1[:])
            v1_t = cp.tile([P, KG1 * CH], BF)
            nc.sync.dma_start(v1_t[:], v1[:])
            xT_t = cp.tile([P, KG1 * NPAD], BF)
            nc.sync.dma_start(xT_t[:], xT[:])
            ball_t = cp.tile([P, NLAYERS * CH], F32)
            nc.sync.dma_start(ball_t[:], ball[:])
            wk_t = cp.tile([P, 6 * KGC * CH], BF)
            nc.sync.dma_start(wk_t[:], wk[:])
            vk_t = cp.tile([P, 6 * KGC * CH], BF)
            wd_t = cp.tile([P, KGC * N_LABELS], BF)
            bdr_t = cp.tile([P, N_LABELS], F32)
            ident = cp.tile([P, P], BF)
            make_identity(nc, ident[:])

            def scatter_phase(width, gsrc_full, gsrc_pref, rs_out, ptag,
                              mid_cb=None):
                """Grouped scatter.  Per chunk: first queue EVERY group's
                early-tile gathers (rows < EROW, via the prefix-slice AP,
                so they depend only on the early m-table blocks and can run
                under the previous layer's trailing collective), then per
                group stream the sel columns, gather the late tiles, and
                scatter-add into each block's accumulator.  After a chunk's
                partial writes, launch its ReduceScatter chunk.  mid_cb
                (the chunk-0 epilogue) is emitted near the end of chunk 1
                so its early m-table writes land before that collective
                finishes."""
                LOOKAHEAD = 6
                for ci, subids in enumerate(CHUNKS):
                    partial = dram.tile([NCORES, CHUNK_NB[ci] * P, width], BF,
                                        tag=f"{ptag}c{ci}")
                    seq = [cg * NSG + si
                           for cg in range(NCORES) for si in subids]
                    # fewest-late-tiles first: extends the early runway at
                    # the layer boundary before the first late-gather wait
                    seq.sort(key=lambda g: ntiles_a[g] - ntA[g])
                    gmsgs = {}

                    def emit_early(g):
                        co = int(colof[g])
                        msgs = []
                        for t in range(int(ntA[g])):
                            msg = msgp.tile([P, CH], BF, tag="msg")
                            nc.gpsimd.indirect_dma_start(
                                out=msg[:, :width], out_offset=None,
                                in_=gsrc_pref,
                                in_offset=bass.IndirectOffsetOnAxis(
                                    ap=idx_t[:, co + t:co + t + 1], axis=0))
                            msgs.append(msg)
                        gmsgs[g] = msgs

                    for k in range(min(LOOKAHEAD, len(seq))):
                        emit_early(seq[k])
                    for k, g in enumerate(seq):
                        if (ci == len(CHUNKS) - 1
                                and k == len(seq) - 1 - MID_BACK
                                and mid_cb is not None):
                            mid_cb()
                        if True:
                            cg, si = g // NSG, g % NSG
                            co, cz = int(colof[g]), int(czof[g])
                            ntg, S = int(ntiles_a[g]), SUBSZ[si]
                            selg = selp.tile([P, ntmax * SMAX * P], BF,
                                             tag="selg")
                            nc.scalar.dma_start(
                                selg[:, :ntg * S * P],
                                sel[:, cz * P:(cz + ntg * S) * P])
                            if k + LOOKAHEAD < len(seq):
                                emit_early(seq[k + LOOKAHEAD])
                            msgs = gmsgs[g]
                            for t in range(int(ntA[g]), ntg):
                                msg = msgp.tile([P, CH], BF, tag="msg")
                                nc.gpsimd.indirect_dma_start(
                                    out=msg[:, :width], out_offset=None,
                                    in_=gsrc_full,
                                    in_offset=bass.IndirectOffsetOnAxis(
                                        ap=idx_t[:, co + t:co + t + 1], axis=0))
                                msgs.append(msg)
                            row0 = (FIRSTB[si] - CHUNK_R0[ci]) * P
                            for j in range(S):
                                acc = psagg.tile([P, CH], F32, tag="agg")
                                for t in range(ntg):
                                    nc.tensor.matmul(
                                        acc[:, :width],
                                        selg[:, (t * S + j) * P:(t * S + j + 1) * P],
                                        msgs[t][:, :width],
                                        start=(t == 0), stop=(t == ntg - 1))
                                pwt = pwp.tile([P, CH], BF, tag="pw")
                                nc.vector.tensor_copy(pwt[:, :width],
                                                      acc[:, :width])
                                nc.sync.dma_start(
                                    partial[cg, row0 + j * P:
                                            row0 + (j + 1) * P, :],
                                    pwt[:, :width])
                    nc.gpsimd.collective_compute(
                        "ReduceScatter", mybir.AluOpType.add,
                        replica_groups=[list(range(NCORES))],
                        ins=[partial[:].opt()],
                        outs=[rs_out[CHUNK_R0[ci] * P:
                                     (CHUNK_R0[ci] + CHUNK_NB[ci]) * P,
                                     :].opt()])

            for rep in range(repeat):
                # ================= layer 1 (x-form) =================
                # hv1 = x @ V1 + b1, precomputed to SBUF (overlaps scatter/RS)
                hv_sb = []
                for b in range(NBL):
                    hps = psm.tile([P, CH], F32, tag="m")
                    for g in range(KG1):
                        nc.tensor.matmul(
                            hps[:],
                            xT_t[:, g * NPAD + b * P:g * NPAD + (b + 1) * P],
                            v1_t[:, g * CH:(g + 1) * CH],
                            start=(g == 0), stop=(g == KG1 - 1))
                    hvb = hvp.tile([P, CH], BF, tag="hv")
                    nc.vector.tensor_add(hvb[:], hps[:], ball_t[:, 0:CH])
                    hv_sb.append(hvb)

                rs1 = dram.tile([NPAD, IN_F], BF, tag=f"rs0_{rep}")

                def emit_m(hT, b, l_next, mtab_next):
                    """m = h @ W for node block b of the next conv layer,
                    written into its m-table half (A for blocks 0-6, B for
                    7-9, so the next layer's early gathers can start while
                    the trailing ReduceScatter chunk still runs)."""
                    wt = wk_t[:, (l_next - 1) * KGC * CH:l_next * KGC * CH]
                    mps = psm.tile([P, CH], F32, tag="m")
                    for g in range(KGC):
                        nc.tensor.matmul(
                            mps[:],
                            hT[:, g * NPAD + b * P:g * NPAD + (b + 1) * P],
                            wt[:, g * CH:(g + 1) * CH],
                            start=(g == 0), stop=(g == KGC - 1))
                    m_bf = mp.tile([P, CH], BF, tag="mbf")
                    nc.scalar.activation(
                        m_bf[:], mps[:], mybir.ActivationFunctionType.Copy)
                    nc.scalar.dma_start(
                        mtab_next[b * P:(b + 1) * P, :], m_bf[:])

                def emit_final(hT, b):
                    o_sb = op.tile([P, N_LABELS], F32, tag="o")
                    fps = []
                    for c in range(3):
                        fin_ps = psagg.tile([P, FIN_CHUNK], F32, tag="agg")
                        fps.append(fin_ps)
                    for g in range(KGC):
                        for c in range(3):
                            nc.tensor.matmul(
                                fps[c][:],
                                hT[:, g * NPAD + b * P:g * NPAD + (b + 1) * P],
                                wd_t[:, g * N_LABELS + c * FIN_CHUNK:
                                     g * N_LABELS + (c + 1) * FIN_CHUNK],
                                start=(g == 0), stop=(g == KGC - 1))
                    for c in range(3):
                        sl = slice(c * FIN_CHUNK, (c + 1) * FIN_CHUNK)
                        nc.vector.tensor_add(fps[c][:], fps[c][:], bdr_t[:, sl])
                        nc.scalar.activation(
                            o_sb[:, sl], fps[c][:],
                            mybir.ActivationFunctionType.Copy)
                    if rep == repeat - 1:
                        nc.sync.dma_start(out[b * P:(b + 1) * P, :], o_sb[:])

                def alloc_mtab(l):
                    mtab_n = dram.tile([NPAD, CH], BF, tag=f"mt{l % 2}")
                    return mtab_n

                hT_cur = htp.tile([P, KGC * NPAD], BF, tag="hT")
                mtab_next = alloc_mtab(1)

                def epi1_block(b):
                    u_sb = aggp.tile([P, CH], BF, tag="aggs")
                    nc.scalar.dma_start(u_sb[:, :IN_F],
                                        rs1[b * P:(b + 1) * P, :])
                    utt = utp.tile([P, IN_F], BF, tag="ut")
                    for g in range(KG1):
                        tr = pstr.tile([P, P], BF, tag="tr")
                        nc.tensor.transpose(
                            tr[:], u_sb[:, g * P:(g + 1) * P], ident[:])
                        nc.vector.tensor_copy(utt[:, g * P:(g + 1) * P], tr[:])
                    ups = psm.tile([P, CH], F32, tag="m")
                    for g in range(KG1):
                        nc.tensor.matmul(
                            ups[:], utt[:, g * P:(g + 1) * P],
                            w1_t[:, g * CH:(g + 1) * CH],
                            start=(g == 0), stop=(g == KG1 - 1))
                    ep = epp.tile([P, CH], BF, tag="ep")
                    nc.vector.tensor_add(ep[:], ups[:], hv_sb[b][:])
                    h_bf = hp.tile([P, CH], BF, tag="h")
                    nc.scalar.activation(
                        h_bf[:], ep[:], mybir.ActivationFunctionType.Relu)
                    for cg in range(KGC):
                        tr2 = pstr.tile([P, P], BF, tag="tr")
                        nc.tensor.transpose(
                            tr2[:], h_bf[:, cg * P:(cg + 1) * P], ident[:])
                        nc.vector.tensor_copy(
                            hT_cur[:, cg * NPAD + b * P:cg * NPAD + (b + 1) * P],
                            tr2[:])
                    emit_m(hT_cur, b, 1, mtab_next)

                scatter_phase(IN_F, xrows[:], xrows[:EROW], rs1, f"ptx{rep}")
                if rep == 0:
                    nc.sync.dma_start(vk_t[:], vk[:])
                    nc.sync.dma_start(wd_t[:], wd[:])
                    nc.sync.dma_start(bdr_t[:], bdr[:])
                for b in range(NBL):
                    epi1_block(b)

                # ================= layers 2..7 (m-form) =================
                for l in range(1, NLAYERS):
                    vt = vk_t[:, (l - 1) * KGC * CH:l * KGC * CH]
                    mtab = mtab_next
                    rs = dram.tile([NPAD, CH], BF, tag=f"rs{l}_{rep}")
                    if l < NLAYERS - 1:
                        mtab_next = alloc_mtab(l + 1)
                    hT_next = htp.tile([P, KGC * NPAD], BF, tag="hT")

                    # epilogue: per block hv, add+relu, transposes, then the
                    # next layer's m (or the final dense) for that block;
                    # chunk-0 blocks interleave with the scatter tail so
                    # m-table A lands before the trailing collective
                    hvbs = {}

                    def hv_block(b, l=l, vt=vt, hT_cur=hT_cur):
                        hps = psm.tile([P, CH], F32, tag="m")
                        for g in range(KGC):
                            nc.tensor.matmul(
                                hps[:],
                                hT_cur[:, g * NPAD + b * P:g * NPAD + (b + 1) * P],
                                vt[:, g * CH:(g + 1) * CH],
                                start=(g == 0), stop=(g == KGC - 1))
                        hvb = hvp.tile([P, CH], BF, tag="hv")
                        nc.vector.tensor_add(
                            hvb[:], hps[:], ball_t[:, l * CH:(l + 1) * CH])
                        hvbs[b] = hvb

                    def epi_rest(b, l=l, rs=rs, hT_next=hT_next,
                                 mtab_next=mtab_next):
                        agg_sb = aggp.tile([P, CH], BF, tag="aggs")
                        nc.scalar.dma_start(agg_sb[:], rs[b * P:(b + 1) * P, :])
                        ep = epp.tile([P, CH], BF, tag="ep")
                        nc.vector.tensor_add(ep[:], agg_sb[:], hvbs[b][:])
                        h_bf = hp.tile([P, CH], BF, tag="h")
                        nc.scalar.activation(
                            h_bf[:], ep[:], mybir.ActivationFunctionType.Relu)
                        for cg in range(KGC):
                            tr2 = pstr.tile([P, P], BF, tag="tr")
                            nc.tensor.transpose(
                                tr2[:], h_bf[:, cg * P:(cg + 1) * P], ident[:])
                            nc.vector.tensor_copy(
                                hT_next[:, cg * NPAD + b * P:cg * NPAD + (b + 1) * P],
                                tr2[:])
                        if l < NLAYERS - 1:
                            emit_m(hT_next, b, l + 1, mtab_next)
                        else:
                            emit_final(hT_next, b)

                    scatter_phase(CH, mtab[:], mtab[:EROW], rs,
                                  f"pt{l % 2}_{rep}",
                                  mid_cb=lambda hv_block=hv_block:
                                      [hv_block(b) for b in range(NBL)])
                    for b in range(NBL):
                        epi_rest(b)
                    hT_cur = hT_next

    _split_excess_waits(nc)
    return nc


# ------------------------------------------------------------- entry point
def kernel(x, src, dst, W1, V1, b1, Wk, Vk, bk, Wd, bd, _repeat=1, _nc_cache={}):
    x = np.asarray(x, np.float32)
    ntiles, idx_tabs, sel_tabs = _prep_edges(src, dst)

    key = (tuple(ntiles), _repeat)
    if key not in _nc_cache:
        _nc_cache[key] = _build(ntiles, repeat=_repeat)
    nc = _nc_cache[key]

    # weights (replicated, host-packed)
    w1p = _pack_rhs(np.asarray(W1, np.float32), KG1, CH).astype(BFNP)
    v1p = _pack_rhs(np.asarray(V1, np.float32), KG1, CH).astype(BFNP)
    wkp = np.concatenate(
        [_pack_rhs(np.asarray(Wk[i], np.float32), KGC, CH) for i in range(6)],
        axis=1).astype(BFNP)
    vkp = np.concatenate(
        [_pack_rhs(np.asarray(Vk[i], np.float32), KGC, CH) for i in range(6)],
        axis=1).astype(BFNP)
    wdp = _pack_rhs(np.asarray(Wd, np.float32), KGC, N_LABELS).astype(BFNP)
    ballv = np.concatenate(
        [np.asarray(b1, np.float32)] + [np.asarray(bk[i], np.float32)
                                        for i in range(6)])
    ballp = np.broadcast_to(ballv, (P, NLAYERS * CH)).copy()
    bdp = np.broadcast_to(np.asarray(bd, np.float32), (P, N_LABELS)).copy()

    in_maps = []
    for p in range(NCORES):
        xp = np.zeros((NPAD, IN_F), np.float32)
        xp[:NPC] = x[p * NPC:(p + 1) * NPC]
        xTp = _pack_lhsT(np.ascontiguousarray(xp.T), KG1).astype(BFNP)
        xbf = xp.astype(BFNP)
        in_maps.append({
            "xT": xTp, "xrows": xbf,
            "idx": idx_tabs[p], "sel": sel_tabs[p],
            "w1": w1p, "v1": v1p, "wk": wkp, "vk": vkp, "wd": wdp,
            "ball": ballp, "bdr": bdp,
        })

    res = run_bass_kernel_spmd(nc, in_maps, core_ids=list(range(NCORES)))
    outp = np.empty((N_NODES, N_LABELS), np.float32)
    for p in range(NCORES):
        outp[p * NPC:(p + 1) * NPC] = res.results[p]["out"][:NPC]
    return outp
